# revision 21
# baseline (speedup 1.0000x reference)
"""Trainium2 Bass kernel for a 3D attention block (GroupNorm -> 1x1 conv ->
4-head attention over 4096 tokens -> out-proj -> residual).

Sharding: batch(2) x heads(4) = 8 (b, h) pairs, one per NeuronCore.
Each core computes, for its (b, h):
    hn = GroupNorm(x[b]); h = W_in @ hn + b_in
    q = 0.125*(Wq_h @ h + bq_h); k = Wk_h @ h + bk_h; v = Wv_h @ h + bv_h
    S^T = k^T q (per 128-j chunk);  P = exp(S^T);  out = (P^T-contracted) v
    y_part = Wout[:, h] @ (out / rowsum)
Host sums the 4 per-head partials per batch and adds b_out + x (the unshard
step). All weights are pre-sliced/pre-transposed per core on the host.

Layouts on device (partition dim first):
    x, hn, h  : 2 chunks of (128 ch, 4096 tok)
    q, k, v   : (64 d, 4096 tok)
    vT        : (128 j, 32 chunk, 65) with col 64 = ones (softmax denominator)
    S^T tiles : psum (128 j, 1024 i) = 2 j-chunks side by side
    out       : psum (65 d', 512 i) accumulated over 32 j-chunks
"""

import numpy as np
from contextlib import ExitStack

import concourse.bass as bass
import concourse.tile as tile
from concourse import mybir
from concourse.bass_utils import run_bass_kernel_spmd

F32 = mybir.dt.float32
AF = mybir.ActivationFunctionType
OP = mybir.AluOpType

P = 128
C = 256
HDIM = 64
NTOK = 4096
FT = 512               # matmul moving free dim (fp32 psum bank)
NI = NTOK // FT        # 8 i-tiles
NJ = NTOK // P         # 32 j-chunks
PAIR = 2               # j-chunks per score psum tile (2 banks)
NG = NJ // PAIR        # 16 groups per i-tile
EPS = 1e-5


def _emit(ctx: ExitStack, tc: tile.TileContext, d):
    nc = tc.nc
    # fp32 matmuls run at 4 cycles/column; float32r (same bits, different PE
    # datapath) runs at 1 cycle/column when the moving free dim is >= 256.
    r = lambda ap: ap.bitcast(mybir.dt.float32r)

    const = ctx.enter_context(tc.tile_pool(name="const", bufs=1))
    data = ctx.enter_context(tc.tile_pool(name="data", bufs=1))
    sm = ctx.enter_context(tc.tile_pool(name="sm", bufs=2))

    # ---- constant loads -------------------------------------------------
    def cload(tag, shape, src):
        t = const.tile(shape, F32, tag=tag)
        nc.sync.dma_start(out=t, in_=src[:])
        return t

    def wload(tag, shape, src):
        # Matmul weights are staged through a DVE copy: a matmul (LDWEIGHTS)
        # can carry only ONE hw sync-wait, so its operands must not depend on
        # two different engines (DMA + compute). After staging, every matmul
        # weight is DVE-produced.
        stage = cload(tag + "_st", shape, src)
        t = const.tile(shape, F32, tag=tag, name=tag)
        nc.vector.tensor_copy(out=t.bitcast(mybir.dt.float32r), in_=stage)
        return t

    def wload_f32(tag, shape, src):
        stage = cload(tag + "_st", shape, src)
        t = const.tile(shape, F32, tag=tag, name=tag)
        nc.vector.tensor_copy(out=t, in_=stage)
        return t

    x = [data.tile([P, NTOK], F32, tag=f"x{c}", name=f"x{c}") for c in range(2)]
    for c in range(2):
        for w4 in range(4):
            nc.sync.dma_start(out=x[c][:, w4 * 1024:(w4 + 1) * 1024],
                              in_=d["x"][c * P:(c + 1) * P, w4 * 1024:(w4 + 1) * 1024])

    winT = [wload(f"winT{c}", [P, C], d["winT"][c * P:(c + 1) * P, :]) for c in range(2)]
    wqT = [wload(f"wqT{c}", [P, HDIM], d["wqT"][c * P:(c + 1) * P, :]) for c in range(2)]
    wkT = [wload(f"wkT{c}", [P, HDIM], d["wkT"][c * P:(c + 1) * P, :]) for c in range(2)]
    wvT = [wload(f"wvT{c}", [P, HDIM], d["wvT"][c * P:(c + 1) * P, :]) for c in range(2)]
    woT = wload("woT", [HDIM, C], d["woT"])
    b_in = [cload(f"bin{c}", [P, 1], d["b_in"][c * P:(c + 1) * P, :]) for c in range(2)]
    bq = cload("bq", [HDIM, 1], d["bq"])
    bk = cload("bk", [HDIM, 1], d["bk"])
    bv = cload("bv", [HDIM, 1], d["bv"])
    gnw = [cload(f"gnw{c}", [P, 1], d["gnw"][c * P:(c + 1) * P, :]) for c in range(2)]
    gnb = [cload(f"gnb{c}", [P, 1], d["gnb"][c * P:(c + 1) * P, :]) for c in range(2)]
    G = wload_f32("G", [P, 16], d["G"])
    GT = wload_f32("GT", [16, P], d["GT"])
    ident = wload_f32("ident", [HDIM, HDIM], d["ident"])
    eps16 = const.tile([16, 1], F32, tag="eps16", name="eps16")
    nc.vector.memset(eps16, EPS)
    ones64 = const.tile([1, HDIM], F32, tag="ones64", name="ones64")
    nc.vector.memset(ones64, 1.0)
    ones_col = const.tile([P, 1], mybir.dt.bfloat16, tag="ones_col", name="ones_col")
    nc.vector.memset(ones_col, 1.0)


    hn = [data.tile([P, NTOK], F32, tag=f"hn{c}", name=f"hn{c}") for c in range(2)]
    h = [data.tile([P, NTOK], F32, tag=f"h{c}", name=f"h{c}") for c in range(2)]
    q = data.tile([HDIM, NTOK], mybir.dt.bfloat16, tag="q", name="q")
    k = data.tile([HDIM, NTOK], mybir.dt.bfloat16, tag="k", name="k")
    v = data.tile([HDIM, NTOK], F32, tag="v", name="v")
    vT = data.tile([P, NJ, HDIM + 1], mybir.dt.bfloat16, tag="vT", name="vT")

    # ---- prologue psum pools (close before attention) -------------------
    with tc.tile_pool(name="ps_mm", bufs=2, space="PSUM") as ps_mm, \
         tc.tile_pool(name="ps_tr", bufs=2, space="PSUM") as ps_tr, \
         tc.tile_pool(name="ps_st", bufs=4, space="PSUM") as ps_st:

        # ---- GroupNorm ----------------------------------------------------
        for c in range(2):
            stats8 = sm.tile([P, 8, 6], F32, tag="stats8", name="stats8")
            for s in range(8):
                nc.vector.bn_stats(out=stats8[:, s, :], in_=x[c][:, s * FT:(s + 1) * FT])
            mv = sm.tile([P, 2], F32, tag="mv", name="mv")
            nc.vector.bn_aggr(out=mv, in_=stats8)
            # stat2 = [mu_c, E[x^2]_c]
            stat2 = sm.tile([P, 2], F32, tag="stat2", name="stat2")
            nc.vector.tensor_copy(out=stat2[:, 0:1], in_=mv[:, 0:1])
            nc.vector.tensor_mul(out=stat2[:, 1:2], in0=mv[:, 0:1], in1=mv[:, 0:1])
            nc.vector.tensor_add(out=stat2[:, 1:2], in0=stat2[:, 1:2], in1=mv[:, 1:2])
            # group sums (16 groups per chunk)
            ps_g = ps_st.tile([P, 2], F32, tag="st", name="sg")
            nc.tensor.matmul(ps_g[0:16, :], lhsT=G, rhs=stat2, start=True, stop=True)
            sgx = sm.tile([16, 2], F32, tag="sgx", name="sgx")
            nc.vector.tensor_scalar_mul(out=sgx, in0=ps_g[0:16, :], scalar1=0.125)  # /8
            musqg = sm.tile([16, 1], F32, tag="musqg", name="musqg")
            nc.vector.tensor_mul(out=musqg, in0=sgx[:, 0:1], in1=sgx[:, 0:1])
            varg = sm.tile([16, 1], F32, tag="varg", name="varg")
            nc.vector.tensor_tensor(out=varg, in0=sgx[:, 1:2], in1=musqg, op=OP.subtract)
            sd = sm.tile([16, 1], F32, tag="sd", name="sd")
            nc.scalar.activation(out=sd, in_=varg, func=AF.Sqrt, bias=eps16)
            rstd = sm.tile([16, 1], F32, tag="rstd", name="rstd")
            nc.vector.reciprocal(out=rstd, in_=sd)
            gr = sm.tile([16, 2], F32, tag="gr", name="gr")
            nc.vector.tensor_copy(out=gr[:, 0:1], in_=sgx[:, 0:1])
            nc.vector.tensor_copy(out=gr[:, 1:2], in_=rstd)
            ps_ch = ps_st.tile([P, 2], F32, tag="st", name="sch")
            nc.tensor.matmul(ps_ch, lhsT=GT, rhs=gr, start=True, stop=True)
            A = sm.tile([P, 1], F32, tag="A", name="A")
            nc.vector.tensor_mul(out=A, in0=ps_ch[:, 1:2], in1=gnw[c])
            tmp = sm.tile([P, 1], F32, tag="tmp", name="tmp")
            nc.vector.tensor_mul(out=tmp, in0=ps_ch[:, 0:1], in1=A)
            Bv = sm.tile([P, 1], F32, tag="Bv", name="Bv")
            nc.vector.tensor_tensor(out=Bv, in0=gnb[c], in1=tmp, op=OP.subtract)
            nc.vector.tensor_scalar(out=r(hn[c]), in0=x[c], scalar1=A, scalar2=Bv,
                                    op0=OP.mult, op1=OP.add)

        # ---- h = W_in @ hn + b_in ----------------------------------------
        for oc in range(2):
            for it in range(NI):
                ps = ps_mm.tile([P, FT], F32, tag="mm", name="mm")
                for cc in range(2):
                    nc.tensor.matmul(ps, lhsT=r(winT[cc][:, oc * P:(oc + 1) * P]),
                                     rhs=r(hn[cc][:, it * FT:(it + 1) * FT]),
                                     start=(cc == 0), stop=(cc == 1))
                nc.vector.tensor_scalar_add(out=r(h[oc][:, it * FT:(it + 1) * FT]),
                                            in0=ps, scalar1=b_in[oc])

        # ---- q, k, v -------------------------------------------------------
        for dst, wT, bias in ((q, wqT, bq), (k, wkT, bk), (v, wvT, bv)):
            for it in range(NI):
                ps = ps_mm.tile([P, FT], F32, tag="mm", name="mm")
                for cc in range(2):
                    nc.tensor.matmul(ps[0:HDIM, :], lhsT=r(wT[cc]),
                                     rhs=r(h[cc][:, it * FT:(it + 1) * FT]),
                                     start=(cc == 0), stop=(cc == 1))
                nc.vector.tensor_scalar_add(out=dst[:, it * FT:(it + 1) * FT],
                                            in0=ps[0:HDIM, :], scalar1=bias)

        # ---- vT (with ones column for softmax denominators) ---------------
        nc.vector.tensor_copy(out=vT[:, :, HDIM:HDIM + 1],
                              in_=ones_col.to_broadcast([P, NJ, 1]))
        for jc in range(NJ):
            ps = ps_tr.tile([P, HDIM], F32, tag="tr", name="tr")
            nc.tensor.transpose(out=ps, in_=v[:, jc * P:(jc + 1) * P], identity=ident)
            nc.vector.tensor_copy(out=vT[:, jc, 0:HDIM], in_=ps)

    # ---- attention ------------------------------------------------------
    tc.strict_bb_all_engine_barrier()
    # Absorb cross-engine waits on a PE nop: the first post-barrier matmul
    # may otherwise need >1 hw sync-wait, which LDWEIGHTS cannot encode.
    with tc.tile_critical():
        pe_nop = nc.tensor.nop(hint="dep").ins
        pe_nop.ins = [nc.tensor.lower_ap(vT[:, 0, :]),
                      nc.tensor.lower_ap(q[:, 0:FT]),
                      nc.tensor.lower_ap(k[:, 0:P])]
    qk_ps = ctx.enter_context(tc.tile_pool(name="qk_ps", bufs=2, space="PSUM"))
    pv_ps = ctx.enter_context(tc.tile_pool(name="pv_ps", bufs=1, space="PSUM"))
    wb_ps = ctx.enter_context(tc.tile_pool(name="wb_ps", bufs=1, space="PSUM"))
    es_pool = ctx.enter_context(tc.tile_pool(name="es", bufs=3))

    AFT = 1024
    for it in range(NTOK // AFT):
        isl = [slice(it * AFT + hf * FT, it * AFT + (hf + 1) * FT) for hf in range(2)]
        pv = pv_ps.tile([HDIM + 1, AFT], F32, tag="pv", name="pv")
        for jc in range(NJ):
            qk = qk_ps.tile([P, AFT], F32, tag="qk", name="qk")
            for hf in range(2):
                nc.tensor.matmul(qk[:, hf * FT:(hf + 1) * FT],
                                 lhsT=k[:, jc * P:(jc + 1) * P], rhs=q[:, isl[hf]],
                                 start=True, stop=True)
            es = es_pool.tile([P, AFT], mybir.dt.bfloat16, tag="es", name="es")
            nc.scalar.activation(out=es, in_=qk, func=AF.Exp)
            for hf in range(2):
                nc.tensor.matmul(pv[:, hf * FT:(hf + 1) * FT],
                                 lhsT=vT[:, jc, :], rhs=es[:, hf * FT:(hf + 1) * FT],
                                 start=(jc == 0), stop=(jc == NJ - 1),
                                 skip_group_check=True)
        # normalize: out[d, i] / out[64, i]; broadcast recip across partitions
        # via a rank-1 matmul (ones column x recip row)
        recip = sm.tile([1, AFT], F32, tag="recip", name="recip")
        nc.vector.reciprocal(out=recip, in_=pv[HDIM:HDIM + 1, :])
        bc = sm.tile([HDIM, AFT], F32, tag="bc_sb", name="bc_sb")
        for hf in range(2):
            bc_ps = wb_ps.tile([HDIM, FT], F32, tag="bc", name="bc")
            nc.tensor.matmul(bc_ps, lhsT=ones64,
                             rhs=recip[:, hf * FT:(hf + 1) * FT],
                             start=True, stop=True)
            nc.vector.tensor_copy(out=bc[:, hf * FT:(hf + 1) * FT], in_=bc_ps)
        onorm = sm.tile([HDIM, AFT], F32, tag="onorm", name="onorm")
        nc.vector.tensor_mul(out=r(onorm), in0=pv[0:HDIM, :], in1=bc)
        # y_part = Wout_h @ onorm
        for oc in range(2):
            for hf in range(2):
                wp = wb_ps.tile([P, FT], F32, tag="wout", name="wout")
                nc.tensor.matmul(wp, lhsT=r(woT[:, oc * P:(oc + 1) * P]),
                                 rhs=r(onorm[:, hf * FT:(hf + 1) * FT]),
                                 start=True, stop=True)
                y_sb = sm.tile([P, FT], F32, tag="y_sb", name="y_sb", bufs=4)
                nc.vector.tensor_copy(out=y_sb, in_=wp)
                nc.sync.dma_start(
                    out=d["y"][oc * P:(oc + 1) * P,
                               it * AFT + hf * FT:it * AFT + (hf + 1) * FT],
                    in_=y_sb)


def _build_nc():
    nc = bass.Bass()
    d = {
        "x": nc.dram_tensor("x", [C, NTOK], F32, kind="ExternalInput"),
        "winT": nc.dram_tensor("winT", [C, C], F32, kind="ExternalInput"),
        "b_in": nc.dram_tensor("b_in", [C, 1], F32, kind="ExternalInput"),
        "wqT": nc.dram_tensor("wqT", [C, HDIM], F32, kind="ExternalInput"),
        "bq": nc.dram_tensor("bq", [HDIM, 1], F32, kind="ExternalInput"),
        "wkT": nc.dram_tensor("wkT", [C, HDIM], F32, kind="ExternalInput"),
        "bk": nc.dram_tensor("bk", [HDIM, 1], F32, kind="ExternalInput"),
        "wvT": nc.dram_tensor("wvT", [C, HDIM], F32, kind="ExternalInput"),
        "bv": nc.dram_tensor("bv", [HDIM, 1], F32, kind="ExternalInput"),
        "woT": nc.dram_tensor("woT", [HDIM, C], F32, kind="ExternalInput"),
        "gnw": nc.dram_tensor("gnw", [C, 1], F32, kind="ExternalInput"),
        "gnb": nc.dram_tensor("gnb", [C, 1], F32, kind="ExternalInput"),
        "G": nc.dram_tensor("G", [P, 16], F32, kind="ExternalInput"),
        "GT": nc.dram_tensor("GT", [16, P], F32, kind="ExternalInput"),
        "ident": nc.dram_tensor("ident", [HDIM, HDIM], F32, kind="ExternalInput"),
        "y": nc.dram_tensor("y", [C, NTOK], F32, kind="ExternalOutput"),
    }
    with tile.TileContext(nc) as tc:
        with ExitStack() as ctx:
            _emit(ctx, tc, d)
    _split_matmul_waits(nc)
    return nc


def _split_matmul_waits(nc):
    """Walrus encodes at most ONE hw sync-wait per engine instruction
    (matmul/LDWEIGHTS, tensor_tensor, ...). Move excess waits onto NoOps
    inserted right before the instruction on the same engine, one wait per
    NoOp; the engine executes them in order, preserving semantics."""
    fixed = 0
    for fn in nc.m.functions:
        for blk in fn.blocks:
            insts = blk.instructions
            out = []
            changed = False
            for inst in insts:
                si = inst.sync_info
                if si is not None and si.on_wait and len(si.on_wait) > 1:
                    waits = list(si.on_wait)
                    for w in waits[:-1]:
                        nop = mybir.InstNoOp(
                            name=f"I-waitsplit-{fixed}", ins=[], outs=[])
                        nop.engine = inst.engine
                        nop.sync_info = mybir.SyncInfo(on_wait=[w], on_update=[])
                        out.append(nop)
                        fixed += 1
                    inst.sync_info = mybir.SyncInfo(
                        on_wait=[waits[-1]], on_update=list(si.on_update or []))
                    changed = True
                out.append(inst)
            if changed:
                blk.instructions = out
    return fixed


_CACHE = {}


def _get_nc():
    if "nc" not in _CACHE:
        _CACHE["nc"] = _build_nc()
    return _CACHE["nc"]


def _make_in_maps(x, gn_w, gn_b, w_in, b_in, w_q, b_q, w_k, b_k, w_v, b_v, w_out):
    f32 = lambda a: np.ascontiguousarray(np.asarray(a), dtype=np.float32)
    x = f32(x)
    Gm = np.zeros((P, 16), np.float32)
    Gm[np.arange(P), np.arange(P) // 8] = 1.0
    common = {
        "winT": f32(np.asarray(w_in).T),
        "b_in": f32(b_in).reshape(C, 1),
        "gnw": f32(gn_w).reshape(C, 1),
        "gnb": f32(gn_b).reshape(C, 1),
        "G": Gm,
        "GT": np.ascontiguousarray(Gm.T),
        "ident": np.eye(HDIM, dtype=np.float32),
    }
    in_maps = []
    for core in range(8):
        b, hd = divmod(core, 4)
        sl = slice(hd * HDIM, (hd + 1) * HDIM)
        m = dict(common)
        m["x"] = f32(x[b].reshape(C, NTOK))
        m["wqT"] = f32((np.asarray(w_q)[sl] * 0.125).T)
        m["bq"] = f32(np.asarray(b_q)[sl] * 0.125).reshape(HDIM, 1)
        m["wkT"] = f32(np.asarray(w_k)[sl].T)
        m["bk"] = f32(np.asarray(b_k)[sl]).reshape(HDIM, 1)
        m["wvT"] = f32(np.asarray(w_v)[sl].T)
        m["bv"] = f32(np.asarray(b_v)[sl]).reshape(HDIM, 1)
        m["woT"] = f32(np.asarray(w_out)[:, sl].T)
        in_maps.append(m)
    return in_maps


def kernel(x, gn_w, gn_b, w_in, b_in, w_q, b_q, w_k, b_k, w_v, b_v, w_out, b_out,
           _trace=False):
    nc = _get_nc()
    in_maps = _make_in_maps(x, gn_w, gn_b, w_in, b_in, w_q, b_q, w_k, b_k,
                            w_v, b_v, w_out)
    res = run_bass_kernel_spmd(nc, in_maps, list(range(8)), trace=_trace)
    parts = np.stack([np.asarray(res.results[i]["y"]) for i in range(8)])
    x_np = np.asarray(x, dtype=np.float32)
    out = (parts.reshape(2, 4, C, NTOK).sum(axis=1)
           + np.asarray(b_out, dtype=np.float32).reshape(1, C, 1)
           + x_np.reshape(2, C, NTOK))
    out = out.reshape(x_np.shape).astype(np.float32)
    if _trace:
        return out, res
    return out


# revision 22
# speedup vs baseline: 1.0473x; 1.0473x over previous
"""Trainium2 Bass kernel for a 3D attention block (GroupNorm -> 1x1 conv ->
4-head attention over 4096 tokens -> out-proj -> residual).

Sharding: batch(2) x heads(4) = 8 (b, h) pairs, one per NeuronCore.
Each core computes, for its (b, h):
    hn = GroupNorm(x[b]); h = W_in @ hn + b_in
    q = 0.125*(Wq_h @ h + bq_h); k = Wk_h @ h + bk_h; v = Wv_h @ h + bv_h
    S^T = k^T q (per 128-j chunk);  P = exp(S^T);  out = (P^T-contracted) v
    y_part = Wout[:, h] @ (out / rowsum)
Host sums the 4 per-head partials per batch and adds b_out + x (the unshard
step). All weights are pre-sliced/pre-transposed per core on the host.

Layouts on device (partition dim first):
    x, hn, h  : 2 chunks of (128 ch, 4096 tok)
    q, k, v   : (64 d, 4096 tok)
    vT        : (128 j, 32 chunk, 65) with col 64 = ones (softmax denominator)
    S^T tiles : psum (128 j, 1024 i) = 2 j-chunks side by side
    out       : psum (65 d', 512 i) accumulated over 32 j-chunks
"""

import numpy as np
from contextlib import ExitStack

import concourse.bass as bass
import concourse.tile as tile
from concourse import mybir
from concourse.bass_utils import run_bass_kernel_spmd

F32 = mybir.dt.float32
AF = mybir.ActivationFunctionType
OP = mybir.AluOpType

P = 128
C = 256
HDIM = 64
NTOK = 4096
FT = 512               # matmul moving free dim (fp32 psum bank)
NI = NTOK // FT        # 8 i-tiles
NJ = NTOK // P         # 32 j-chunks
PAIR = 2               # j-chunks per score psum tile (2 banks)
NG = NJ // PAIR        # 16 groups per i-tile
EPS = 1e-5


def _emit(ctx: ExitStack, tc: tile.TileContext, d):
    nc = tc.nc
    # fp32 matmuls run at 4 cycles/column; float32r (same bits, different PE
    # datapath) runs at 1 cycle/column when the moving free dim is >= 256.
    r = lambda ap: ap.bitcast(mybir.dt.float32r)

    const = ctx.enter_context(tc.tile_pool(name="const", bufs=1))
    data = ctx.enter_context(tc.tile_pool(name="data", bufs=1))
    sm = ctx.enter_context(tc.tile_pool(name="sm", bufs=2))

    # ---- constant loads -------------------------------------------------
    def cload(tag, shape, src):
        t = const.tile(shape, F32, tag=tag)
        nc.sync.dma_start(out=t, in_=src[:])
        return t

    def wload(tag, shape, src):
        # Matmul weights are staged through a DVE copy: a matmul (LDWEIGHTS)
        # can carry only ONE hw sync-wait, so its operands must not depend on
        # two different engines (DMA + compute). After staging, every matmul
        # weight is DVE-produced.
        stage = cload(tag + "_st", shape, src)
        t = const.tile(shape, F32, tag=tag, name=tag)
        nc.vector.tensor_copy(out=t.bitcast(mybir.dt.float32r), in_=stage)
        return t

    def wload_f32(tag, shape, src):
        stage = cload(tag + "_st", shape, src)
        t = const.tile(shape, F32, tag=tag, name=tag)
        nc.vector.tensor_copy(out=t, in_=stage)
        return t

    x = [data.tile([P, NTOK], F32, tag=f"x{c}", name=f"x{c}") for c in range(2)]
    for c in range(2):
        for w4 in range(4):
            nc.sync.dma_start(out=x[c][:, w4 * 1024:(w4 + 1) * 1024],
                              in_=d["x"][c * P:(c + 1) * P, w4 * 1024:(w4 + 1) * 1024])

    winT = [wload(f"winT{c}", [P, C], d["winT"][c * P:(c + 1) * P, :]) for c in range(2)]
    wqT = [wload(f"wqT{c}", [P, HDIM], d["wqT"][c * P:(c + 1) * P, :]) for c in range(2)]
    wkT = [wload(f"wkT{c}", [P, HDIM], d["wkT"][c * P:(c + 1) * P, :]) for c in range(2)]
    wvT = [wload(f"wvT{c}", [P, HDIM], d["wvT"][c * P:(c + 1) * P, :]) for c in range(2)]
    woT = wload("woT", [HDIM, C], d["woT"])
    b_in = [cload(f"bin{c}", [P, 1], d["b_in"][c * P:(c + 1) * P, :]) for c in range(2)]
    bq = cload("bq", [HDIM, 1], d["bq"])
    bk = cload("bk", [HDIM, 1], d["bk"])
    bv = cload("bv", [HDIM, 1], d["bv"])
    gnw = [cload(f"gnw{c}", [P, 1], d["gnw"][c * P:(c + 1) * P, :]) for c in range(2)]
    gnb = [cload(f"gnb{c}", [P, 1], d["gnb"][c * P:(c + 1) * P, :]) for c in range(2)]
    G = wload_f32("G", [P, 16], d["G"])
    GT = wload_f32("GT", [16, P], d["GT"])
    ident = wload_f32("ident", [HDIM, HDIM], d["ident"])
    eps16 = const.tile([16, 1], F32, tag="eps16", name="eps16")
    nc.vector.memset(eps16, EPS)
    ones64 = const.tile([1, HDIM], F32, tag="ones64", name="ones64")
    nc.vector.memset(ones64, 1.0)
    ones_col = const.tile([P, 1], mybir.dt.bfloat16, tag="ones_col", name="ones_col")
    nc.vector.memset(ones_col, 1.0)


    hn = [data.tile([P, NTOK], F32, tag=f"hn{c}", name=f"hn{c}") for c in range(2)]
    h = [data.tile([P, NTOK], F32, tag=f"h{c}", name=f"h{c}") for c in range(2)]
    q = data.tile([HDIM, NTOK], mybir.dt.bfloat16, tag="q", name="q")
    k = data.tile([HDIM, NTOK], mybir.dt.bfloat16, tag="k", name="k")
    v = data.tile([HDIM, NTOK], F32, tag="v", name="v")
    vT = data.tile([P, NJ, HDIM + 1], mybir.dt.bfloat16, tag="vT", name="vT")

    # ---- prologue psum pools (close before attention) -------------------
    with tc.tile_pool(name="ps_mm", bufs=2, space="PSUM") as ps_mm, \
         tc.tile_pool(name="ps_tr", bufs=2, space="PSUM") as ps_tr, \
         tc.tile_pool(name="ps_st", bufs=4, space="PSUM") as ps_st:

        # ---- GroupNorm ----------------------------------------------------
        for c in range(2):
            stats8 = sm.tile([P, 8, 6], F32, tag="stats8", name="stats8")
            for s in range(8):
                nc.vector.bn_stats(out=stats8[:, s, :], in_=x[c][:, s * FT:(s + 1) * FT])
            mv = sm.tile([P, 2], F32, tag="mv", name="mv")
            nc.vector.bn_aggr(out=mv, in_=stats8)
            # stat2 = [mu_c, E[x^2]_c]
            stat2 = sm.tile([P, 2], F32, tag="stat2", name="stat2")
            nc.vector.tensor_copy(out=stat2[:, 0:1], in_=mv[:, 0:1])
            nc.vector.tensor_mul(out=stat2[:, 1:2], in0=mv[:, 0:1], in1=mv[:, 0:1])
            nc.vector.tensor_add(out=stat2[:, 1:2], in0=stat2[:, 1:2], in1=mv[:, 1:2])
            # group sums (16 groups per chunk)
            ps_g = ps_st.tile([P, 2], F32, tag="st", name="sg")
            nc.tensor.matmul(ps_g[0:16, :], lhsT=G, rhs=stat2, start=True, stop=True)
            sgx = sm.tile([16, 2], F32, tag="sgx", name="sgx")
            nc.vector.tensor_scalar_mul(out=sgx, in0=ps_g[0:16, :], scalar1=0.125)  # /8
            musqg = sm.tile([16, 1], F32, tag="musqg", name="musqg")
            nc.vector.tensor_mul(out=musqg, in0=sgx[:, 0:1], in1=sgx[:, 0:1])
            varg = sm.tile([16, 1], F32, tag="varg", name="varg")
            nc.vector.tensor_tensor(out=varg, in0=sgx[:, 1:2], in1=musqg, op=OP.subtract)
            sd = sm.tile([16, 1], F32, tag="sd", name="sd")
            nc.scalar.activation(out=sd, in_=varg, func=AF.Sqrt, bias=eps16)
            rstd = sm.tile([16, 1], F32, tag="rstd", name="rstd")
            nc.vector.reciprocal(out=rstd, in_=sd)
            gr = sm.tile([16, 2], F32, tag="gr", name="gr")
            nc.vector.tensor_copy(out=gr[:, 0:1], in_=sgx[:, 0:1])
            nc.vector.tensor_copy(out=gr[:, 1:2], in_=rstd)
            ps_ch = ps_st.tile([P, 2], F32, tag="st", name="sch")
            nc.tensor.matmul(ps_ch, lhsT=GT, rhs=gr, start=True, stop=True)
            A = sm.tile([P, 1], F32, tag="A", name="A")
            nc.vector.tensor_mul(out=A, in0=ps_ch[:, 1:2], in1=gnw[c])
            tmp = sm.tile([P, 1], F32, tag="tmp", name="tmp")
            nc.vector.tensor_mul(out=tmp, in0=ps_ch[:, 0:1], in1=A)
            Bv = sm.tile([P, 1], F32, tag="Bv", name="Bv")
            nc.vector.tensor_tensor(out=Bv, in0=gnb[c], in1=tmp, op=OP.subtract)
            nc.vector.tensor_scalar(out=r(hn[c]), in0=x[c], scalar1=A, scalar2=Bv,
                                    op0=OP.mult, op1=OP.add)

        # ---- h = W_in @ hn + b_in ----------------------------------------
        for oc in range(2):
            for it in range(NI):
                ps = ps_mm.tile([P, FT], F32, tag="mm", name="mm")
                for cc in range(2):
                    nc.tensor.matmul(ps, lhsT=r(winT[cc][:, oc * P:(oc + 1) * P]),
                                     rhs=r(hn[cc][:, it * FT:(it + 1) * FT]),
                                     start=(cc == 0), stop=(cc == 1))
                nc.vector.tensor_scalar_add(out=r(h[oc][:, it * FT:(it + 1) * FT]),
                                            in0=ps, scalar1=b_in[oc])

        # ---- q, k, v -------------------------------------------------------
        for dst, wT, bias in ((q, wqT, bq), (k, wkT, bk), (v, wvT, bv)):
            for it in range(NI):
                ps = ps_mm.tile([P, FT], F32, tag="mm", name="mm")
                for cc in range(2):
                    nc.tensor.matmul(ps[0:HDIM, :], lhsT=r(wT[cc]),
                                     rhs=r(h[cc][:, it * FT:(it + 1) * FT]),
                                     start=(cc == 0), stop=(cc == 1))
                nc.vector.tensor_scalar_add(out=dst[:, it * FT:(it + 1) * FT],
                                            in0=ps[0:HDIM, :], scalar1=bias)

        # ---- vT (with ones column for softmax denominators) ---------------
        nc.vector.tensor_copy(out=vT[:, :, HDIM:HDIM + 1],
                              in_=ones_col.to_broadcast([P, NJ, 1]))
        for jc in range(NJ):
            ps = ps_tr.tile([P, HDIM], F32, tag="tr", name="tr")
            nc.tensor.transpose(out=ps, in_=v[:, jc * P:(jc + 1) * P], identity=ident)
            nc.vector.tensor_copy(out=vT[:, jc, 0:HDIM], in_=ps)

    # ---- attention ------------------------------------------------------
    tc.strict_bb_all_engine_barrier()
    # Absorb cross-engine waits on a PE nop: the first post-barrier matmul
    # may otherwise need >1 hw sync-wait, which LDWEIGHTS cannot encode.
    with tc.tile_critical():
        pe_nop = nc.tensor.nop(hint="dep").ins
        pe_nop.ins = [nc.tensor.lower_ap(vT[:, 0, :]),
                      nc.tensor.lower_ap(q[:, 0:FT]),
                      nc.tensor.lower_ap(k[:, 0:P])]
    qk_ps = ctx.enter_context(tc.tile_pool(name="qk_ps", bufs=2, space="PSUM"))
    pv_ps = ctx.enter_context(tc.tile_pool(name="pv_ps", bufs=2, space="PSUM"))
    wb_ps = ctx.enter_context(tc.tile_pool(name="wb_ps", bufs=1, space="PSUM"))
    es_pool = ctx.enter_context(tc.tile_pool(name="es", bufs=3))

    for it in range(NI):
        isl = slice(it * FT, (it + 1) * FT)
        pv = pv_ps.tile([HDIM + 1, FT], F32, tag="pv", name="pv")
        for g in range(NG):
            qk = qk_ps.tile([P, PAIR * FT], F32, tag="qk", name="qk")
            for u in range(PAIR):
                jc = g * PAIR + u
                nc.tensor.matmul(qk[:, u * FT:(u + 1) * FT],
                                 lhsT=k[:, jc * P:(jc + 1) * P], rhs=q[:, isl],
                                 start=True, stop=True)
            es = es_pool.tile([P, PAIR * FT], mybir.dt.bfloat16, tag="es", name="es")
            nc.scalar.activation(out=es, in_=qk, func=AF.Exp)
            for u in range(PAIR):
                jc = g * PAIR + u
                nc.tensor.matmul(pv, lhsT=vT[:, jc, :], rhs=es[:, u * FT:(u + 1) * FT],
                                 start=(jc == 0), stop=(jc == NJ - 1),
                                 skip_group_check=True)
        # normalize: out[d, i] / out[64, i]
        recip = sm.tile([1, FT], F32, tag="recip", name="recip")
        nc.vector.reciprocal(out=recip, in_=pv[HDIM:HDIM + 1, :])
        bc_ps = wb_ps.tile([HDIM, FT], F32, tag="bc", name="bc")
        nc.tensor.matmul(bc_ps, lhsT=ones64, rhs=recip, start=True, stop=True)
        bc = sm.tile([HDIM, FT], F32, tag="bc_sb", name="bc_sb")
        nc.vector.tensor_copy(out=bc, in_=bc_ps)
        onorm = sm.tile([HDIM, FT], F32, tag="onorm", name="onorm")
        nc.vector.tensor_mul(out=r(onorm), in0=pv[0:HDIM, :], in1=bc)
        # y_part = Wout_h @ onorm
        for oc in range(2):
            wp = wb_ps.tile([P, FT], F32, tag="wout", name="wout")
            nc.tensor.matmul(wp, lhsT=r(woT[:, oc * P:(oc + 1) * P]), rhs=r(onorm),
                             start=True, stop=True)
            y_sb = sm.tile([P, FT], F32, tag="y_sb", name="y_sb", bufs=4)
            nc.vector.tensor_copy(out=y_sb, in_=wp)
            nc.sync.dma_start(out=d["y"][oc * P:(oc + 1) * P, isl], in_=y_sb)


def _build_nc():
    nc = bass.Bass()
    d = {
        "x": nc.dram_tensor("x", [C, NTOK], F32, kind="ExternalInput"),
        "winT": nc.dram_tensor("winT", [C, C], F32, kind="ExternalInput"),
        "b_in": nc.dram_tensor("b_in", [C, 1], F32, kind="ExternalInput"),
        "wqT": nc.dram_tensor("wqT", [C, HDIM], F32, kind="ExternalInput"),
        "bq": nc.dram_tensor("bq", [HDIM, 1], F32, kind="ExternalInput"),
        "wkT": nc.dram_tensor("wkT", [C, HDIM], F32, kind="ExternalInput"),
        "bk": nc.dram_tensor("bk", [HDIM, 1], F32, kind="ExternalInput"),
        "wvT": nc.dram_tensor("wvT", [C, HDIM], F32, kind="ExternalInput"),
        "bv": nc.dram_tensor("bv", [HDIM, 1], F32, kind="ExternalInput"),
        "woT": nc.dram_tensor("woT", [HDIM, C], F32, kind="ExternalInput"),
        "gnw": nc.dram_tensor("gnw", [C, 1], F32, kind="ExternalInput"),
        "gnb": nc.dram_tensor("gnb", [C, 1], F32, kind="ExternalInput"),
        "G": nc.dram_tensor("G", [P, 16], F32, kind="ExternalInput"),
        "GT": nc.dram_tensor("GT", [16, P], F32, kind="ExternalInput"),
        "ident": nc.dram_tensor("ident", [HDIM, HDIM], F32, kind="ExternalInput"),
        "y": nc.dram_tensor("y", [C, NTOK], F32, kind="ExternalOutput"),
    }
    with tile.TileContext(nc) as tc:
        with ExitStack() as ctx:
            _emit(ctx, tc, d)
    _split_matmul_waits(nc)
    return nc


def _split_matmul_waits(nc):
    """Walrus encodes at most ONE hw sync-wait per engine instruction
    (matmul/LDWEIGHTS, tensor_tensor, ...). Move excess waits onto NoOps
    inserted right before the instruction on the same engine, one wait per
    NoOp; the engine executes them in order, preserving semantics."""
    fixed = 0
    for fn in nc.m.functions:
        for blk in fn.blocks:
            insts = blk.instructions
            out = []
            changed = False
            for inst in insts:
                si = inst.sync_info
                if si is not None and si.on_wait and len(si.on_wait) > 1:
                    waits = list(si.on_wait)
                    for w in waits[:-1]:
                        nop = mybir.InstNoOp(
                            name=f"I-waitsplit-{fixed}", ins=[], outs=[])
                        nop.engine = inst.engine
                        nop.sync_info = mybir.SyncInfo(on_wait=[w], on_update=[])
                        out.append(nop)
                        fixed += 1
                    inst.sync_info = mybir.SyncInfo(
                        on_wait=[waits[-1]], on_update=list(si.on_update or []))
                    changed = True
                out.append(inst)
            if changed:
                blk.instructions = out
    return fixed


_CACHE = {}


def _get_nc():
    if "nc" not in _CACHE:
        _CACHE["nc"] = _build_nc()
    return _CACHE["nc"]


def _make_in_maps(x, gn_w, gn_b, w_in, b_in, w_q, b_q, w_k, b_k, w_v, b_v, w_out):
    f32 = lambda a: np.ascontiguousarray(np.asarray(a), dtype=np.float32)
    x = f32(x)
    Gm = np.zeros((P, 16), np.float32)
    Gm[np.arange(P), np.arange(P) // 8] = 1.0
    common = {
        "winT": f32(np.asarray(w_in).T),
        "b_in": f32(b_in).reshape(C, 1),
        "gnw": f32(gn_w).reshape(C, 1),
        "gnb": f32(gn_b).reshape(C, 1),
        "G": Gm,
        "GT": np.ascontiguousarray(Gm.T),
        "ident": np.eye(HDIM, dtype=np.float32),
    }
    in_maps = []
    for core in range(8):
        b, hd = divmod(core, 4)
        sl = slice(hd * HDIM, (hd + 1) * HDIM)
        m = dict(common)
        m["x"] = f32(x[b].reshape(C, NTOK))
        m["wqT"] = f32((np.asarray(w_q)[sl] * 0.125).T)
        m["bq"] = f32(np.asarray(b_q)[sl] * 0.125).reshape(HDIM, 1)
        m["wkT"] = f32(np.asarray(w_k)[sl].T)
        m["bk"] = f32(np.asarray(b_k)[sl]).reshape(HDIM, 1)
        m["wvT"] = f32(np.asarray(w_v)[sl].T)
        m["bv"] = f32(np.asarray(b_v)[sl]).reshape(HDIM, 1)
        m["woT"] = f32(np.asarray(w_out)[:, sl].T)
        in_maps.append(m)
    return in_maps


def kernel(x, gn_w, gn_b, w_in, b_in, w_q, b_q, w_k, b_k, w_v, b_v, w_out, b_out,
           _trace=False):
    nc = _get_nc()
    in_maps = _make_in_maps(x, gn_w, gn_b, w_in, b_in, w_q, b_q, w_k, b_k,
                            w_v, b_v, w_out)
    res = run_bass_kernel_spmd(nc, in_maps, list(range(8)), trace=_trace)
    parts = np.stack([np.asarray(res.results[i]["y"]) for i in range(8)])
    x_np = np.asarray(x, dtype=np.float32)
    out = (parts.reshape(2, 4, C, NTOK).sum(axis=1)
           + np.asarray(b_out, dtype=np.float32).reshape(1, C, 1)
           + x_np.reshape(2, C, NTOK))
    out = out.reshape(x_np.shape).astype(np.float32)
    if _trace:
        return out, res
    return out


# revision 24
# speedup vs baseline: 1.0532x; 1.0056x over previous
"""Trainium2 Bass kernel for a 3D attention block (GroupNorm -> 1x1 conv ->
4-head attention over 4096 tokens -> out-proj -> residual).

Sharding: batch(2) x heads(4) = 8 (b, h) pairs, one per NeuronCore.
Each core computes, for its (b, h):
    hn = GroupNorm(x[b]); h = W_in @ hn + b_in
    q = 0.125*(Wq_h @ h + bq_h); k = Wk_h @ h + bk_h; v = Wv_h @ h + bv_h
    S^T = k^T q (per 128-j chunk);  P = exp(S^T);  out = (P^T-contracted) v
    y_part = Wout[:, h] @ (out / rowsum)
Host sums the 4 per-head partials per batch and adds b_out + x (the unshard
step). All weights are pre-sliced/pre-transposed per core on the host.

Layouts on device (partition dim first):
    x, hn, h  : 2 chunks of (128 ch, 4096 tok)
    q, k, v   : (64 d, 4096 tok)
    vT        : (128 j, 32 chunk, 65) with col 64 = ones (softmax denominator)
    S^T tiles : psum (128 j, 2x512 i) = 2 j-chunks side by side, exp'd by
                one ACT instruction into bf16; QK/PV matmuls in bf16, channel
                matmuls in float32r (both run 1 PE cycle/column vs 4 for fp32)
    out       : psum (65 d', 512 i) accumulated over 32 j-chunks; row 64 is
                the softmax denominator (ones column trick)
"""

import numpy as np
from contextlib import ExitStack

import concourse.bass as bass
import concourse.tile as tile
from concourse import mybir
from concourse.bass_utils import run_bass_kernel_spmd

F32 = mybir.dt.float32
AF = mybir.ActivationFunctionType
OP = mybir.AluOpType

P = 128
C = 256
HDIM = 64
NTOK = 4096
FT = 512               # matmul moving free dim (fp32 psum bank)
NI = NTOK // FT        # 8 i-tiles
NJ = NTOK // P         # 32 j-chunks
PAIR = 2               # j-chunks per score psum tile (2 banks)
NG = NJ // PAIR        # 16 groups per i-tile
EPS = 1e-5


def _emit(ctx: ExitStack, tc: tile.TileContext, d):
    nc = tc.nc
    # fp32 matmuls run at 4 cycles/column; float32r (same bits, different PE
    # datapath) runs at 1 cycle/column when the moving free dim is >= 256.
    r = lambda ap: ap.bitcast(mybir.dt.float32r)

    const = ctx.enter_context(tc.tile_pool(name="const", bufs=1))
    data = ctx.enter_context(tc.tile_pool(name="data", bufs=1))
    sm = ctx.enter_context(tc.tile_pool(name="sm", bufs=2))

    # ---- constant loads -------------------------------------------------
    def cload(tag, shape, src):
        t = const.tile(shape, F32, tag=tag)
        nc.sync.dma_start(out=t, in_=src[:])
        return t

    def wload(tag, shape, src):
        # Matmul weights are staged through a DVE copy: a matmul (LDWEIGHTS)
        # can carry only ONE hw sync-wait, so its operands must not depend on
        # two different engines (DMA + compute). After staging, every matmul
        # weight is DVE-produced.
        stage = cload(tag + "_st", shape, src)
        t = const.tile(shape, F32, tag=tag, name=tag)
        nc.vector.tensor_copy(out=t.bitcast(mybir.dt.float32r), in_=stage)
        return t

    def wload_f32(tag, shape, src):
        stage = cload(tag + "_st", shape, src)
        t = const.tile(shape, F32, tag=tag, name=tag)
        nc.vector.tensor_copy(out=t, in_=stage)
        return t

    x = [data.tile([P, NTOK], F32, tag=f"x{c}", name=f"x{c}") for c in range(2)]
    for c in range(2):
        for w4 in range(4):
            nc.sync.dma_start(out=x[c][:, w4 * 1024:(w4 + 1) * 1024],
                              in_=d["x"][c * P:(c + 1) * P, w4 * 1024:(w4 + 1) * 1024])

    winT = [wload(f"winT{c}", [P, C], d["winT"][c * P:(c + 1) * P, :]) for c in range(2)]
    wqT = [wload(f"wqT{c}", [P, HDIM], d["wqT"][c * P:(c + 1) * P, :]) for c in range(2)]
    wkT = [wload(f"wkT{c}", [P, HDIM], d["wkT"][c * P:(c + 1) * P, :]) for c in range(2)]
    wvT = [wload(f"wvT{c}", [P, HDIM], d["wvT"][c * P:(c + 1) * P, :]) for c in range(2)]
    woT = wload("woT", [HDIM, C], d["woT"])
    b_in = [cload(f"bin{c}", [P, 1], d["b_in"][c * P:(c + 1) * P, :]) for c in range(2)]
    bq = cload("bq", [HDIM, 1], d["bq"])
    bk = cload("bk", [HDIM, 1], d["bk"])
    bv = cload("bv", [HDIM, 1], d["bv"])
    gnw = [cload(f"gnw{c}", [P, 1], d["gnw"][c * P:(c + 1) * P, :]) for c in range(2)]
    gnb = [cload(f"gnb{c}", [P, 1], d["gnb"][c * P:(c + 1) * P, :]) for c in range(2)]
    G = wload_f32("G", [P, 16], d["G"])
    GT = wload_f32("GT", [16, P], d["GT"])
    ident = wload_f32("ident", [HDIM, HDIM], d["ident"])
    eps16 = const.tile([16, 1], F32, tag="eps16", name="eps16")
    nc.vector.memset(eps16, EPS)
    ones64 = const.tile([1, HDIM], F32, tag="ones64", name="ones64")
    nc.vector.memset(ones64, 1.0)
    ones_col = const.tile([P, 1], mybir.dt.bfloat16, tag="ones_col", name="ones_col")
    nc.vector.memset(ones_col, 1.0)


    hn = [data.tile([P, NTOK], F32, tag=f"hn{c}", name=f"hn{c}") for c in range(2)]
    h = [data.tile([P, NTOK], F32, tag=f"h{c}", name=f"h{c}") for c in range(2)]
    q = data.tile([HDIM, NTOK], mybir.dt.bfloat16, tag="q", name="q")
    k = data.tile([HDIM, NTOK], mybir.dt.bfloat16, tag="k", name="k")
    v = data.tile([HDIM, NTOK], F32, tag="v", name="v")
    vT = data.tile([P, NJ, HDIM + 1], mybir.dt.bfloat16, tag="vT", name="vT")

    # ---- prologue psum pools (close before attention) -------------------
    with tc.tile_pool(name="ps_mm", bufs=2, space="PSUM") as ps_mm, \
         tc.tile_pool(name="ps_tr", bufs=2, space="PSUM") as ps_tr, \
         tc.tile_pool(name="ps_st", bufs=4, space="PSUM") as ps_st:

        # ---- GroupNorm ----------------------------------------------------
        for c in range(2):
            stats8 = sm.tile([P, 8, 6], F32, tag="stats8", name="stats8")
            for s in range(8):
                nc.vector.bn_stats(out=stats8[:, s, :], in_=x[c][:, s * FT:(s + 1) * FT])
            mv = sm.tile([P, 2], F32, tag="mv", name="mv")
            nc.vector.bn_aggr(out=mv, in_=stats8)
            # stat2 = [mu_c, E[x^2]_c]
            stat2 = sm.tile([P, 2], F32, tag="stat2", name="stat2")
            nc.vector.tensor_copy(out=stat2[:, 0:1], in_=mv[:, 0:1])
            nc.vector.tensor_mul(out=stat2[:, 1:2], in0=mv[:, 0:1], in1=mv[:, 0:1])
            nc.vector.tensor_add(out=stat2[:, 1:2], in0=stat2[:, 1:2], in1=mv[:, 1:2])
            # group sums (16 groups per chunk)
            ps_g = ps_st.tile([P, 2], F32, tag="st", name="sg")
            nc.tensor.matmul(ps_g[0:16, :], lhsT=G, rhs=stat2, start=True, stop=True)
            sgx = sm.tile([16, 2], F32, tag="sgx", name="sgx")
            nc.vector.tensor_scalar_mul(out=sgx, in0=ps_g[0:16, :], scalar1=0.125)  # /8
            musqg = sm.tile([16, 1], F32, tag="musqg", name="musqg")
            nc.vector.tensor_mul(out=musqg, in0=sgx[:, 0:1], in1=sgx[:, 0:1])
            varg = sm.tile([16, 1], F32, tag="varg", name="varg")
            nc.vector.tensor_tensor(out=varg, in0=sgx[:, 1:2], in1=musqg, op=OP.subtract)
            sd = sm.tile([16, 1], F32, tag="sd", name="sd")
            nc.scalar.activation(out=sd, in_=varg, func=AF.Sqrt, bias=eps16)
            rstd = sm.tile([16, 1], F32, tag="rstd", name="rstd")
            nc.vector.reciprocal(out=rstd, in_=sd)
            gr = sm.tile([16, 2], F32, tag="gr", name="gr")
            nc.vector.tensor_copy(out=gr[:, 0:1], in_=sgx[:, 0:1])
            nc.vector.tensor_copy(out=gr[:, 1:2], in_=rstd)
            ps_ch = ps_st.tile([P, 2], F32, tag="st", name="sch")
            nc.tensor.matmul(ps_ch, lhsT=GT, rhs=gr, start=True, stop=True)
            A = sm.tile([P, 1], F32, tag="A", name="A")
            nc.vector.tensor_mul(out=A, in0=ps_ch[:, 1:2], in1=gnw[c])
            tmp = sm.tile([P, 1], F32, tag="tmp", name="tmp")
            nc.vector.tensor_mul(out=tmp, in0=ps_ch[:, 0:1], in1=A)
            Bv = sm.tile([P, 1], F32, tag="Bv", name="Bv")
            nc.vector.tensor_tensor(out=Bv, in0=gnb[c], in1=tmp, op=OP.subtract)
            for w4 in range(4):
                sl4 = slice(w4 * 1024, (w4 + 1) * 1024)
                nc.vector.tensor_scalar(out=r(hn[c][:, sl4]), in0=x[c][:, sl4],
                                        scalar1=A, scalar2=Bv,
                                        op0=OP.mult, op1=OP.add)

        # ---- h = W_in @ hn + b_in ----------------------------------------
        for oc in range(2):
            for it in range(NI):
                ps = ps_mm.tile([P, FT], F32, tag="mm", name="mm")
                for cc in range(2):
                    nc.tensor.matmul(ps, lhsT=r(winT[cc][:, oc * P:(oc + 1) * P]),
                                     rhs=r(hn[cc][:, it * FT:(it + 1) * FT]),
                                     start=(cc == 0), stop=(cc == 1))
                nc.vector.tensor_scalar_add(out=r(h[oc][:, it * FT:(it + 1) * FT]),
                                            in0=ps, scalar1=b_in[oc])

        # ---- q, k, v -------------------------------------------------------
        for dst, wT, bias in ((q, wqT, bq), (k, wkT, bk), (v, wvT, bv)):
            for it in range(NI):
                ps = ps_mm.tile([P, FT], F32, tag="mm", name="mm")
                for cc in range(2):
                    nc.tensor.matmul(ps[0:HDIM, :], lhsT=r(wT[cc]),
                                     rhs=r(h[cc][:, it * FT:(it + 1) * FT]),
                                     start=(cc == 0), stop=(cc == 1))
                nc.vector.tensor_scalar_add(out=dst[:, it * FT:(it + 1) * FT],
                                            in0=ps[0:HDIM, :], scalar1=bias)

        # ---- vT (with ones column for softmax denominators) ---------------
        nc.vector.tensor_copy(out=vT[:, :, HDIM:HDIM + 1],
                              in_=ones_col.to_broadcast([P, NJ, 1]))
        for jc in range(NJ):
            ps = ps_tr.tile([P, HDIM], F32, tag="tr", name="tr")
            nc.tensor.transpose(out=ps, in_=v[:, jc * P:(jc + 1) * P], identity=ident)
            nc.vector.tensor_copy(out=vT[:, jc, 0:HDIM], in_=ps)

    # ---- attention ------------------------------------------------------
    qk_ps = ctx.enter_context(tc.tile_pool(name="qk_ps", bufs=2, space="PSUM"))
    pv_ps = ctx.enter_context(tc.tile_pool(name="pv_ps", bufs=2, space="PSUM"))
    wb_ps = ctx.enter_context(tc.tile_pool(name="wb_ps", bufs=1, space="PSUM"))
    es_pool = ctx.enter_context(tc.tile_pool(name="es", bufs=4))

    for it in range(NI):
        isl = slice(it * FT, (it + 1) * FT)
        pv = pv_ps.tile([HDIM + 1, FT], F32, tag="pv", name="pv")
        for g in range(NG):
            qk = qk_ps.tile([P, PAIR * FT], F32, tag="qk", name="qk")
            for u in range(PAIR):
                jc = g * PAIR + u
                nc.tensor.matmul(qk[:, u * FT:(u + 1) * FT],
                                 lhsT=k[:, jc * P:(jc + 1) * P], rhs=q[:, isl],
                                 start=True, stop=True)
            es = es_pool.tile([P, PAIR * FT], mybir.dt.bfloat16, tag="es", name="es")
            nc.scalar.activation(out=es, in_=qk, func=AF.Exp)
            for u in range(PAIR):
                jc = g * PAIR + u
                nc.tensor.matmul(pv, lhsT=vT[:, jc, :], rhs=es[:, u * FT:(u + 1) * FT],
                                 start=(jc == 0), stop=(jc == NJ - 1),
                                 skip_group_check=True)
        # normalize: out[d, i] / out[64, i]
        recip = sm.tile([1, FT], F32, tag="recip", name="recip")
        nc.vector.reciprocal(out=recip, in_=pv[HDIM:HDIM + 1, :])
        bc_ps = wb_ps.tile([HDIM, FT], F32, tag="bc", name="bc")
        nc.tensor.matmul(bc_ps, lhsT=ones64, rhs=recip, start=True, stop=True)
        bc = sm.tile([HDIM, FT], F32, tag="bc_sb", name="bc_sb")
        nc.vector.tensor_copy(out=bc, in_=bc_ps)
        onorm = sm.tile([HDIM, FT], F32, tag="onorm", name="onorm")
        nc.vector.tensor_mul(out=r(onorm), in0=pv[0:HDIM, :], in1=bc)
        # y_part = Wout_h @ onorm
        for oc in range(2):
            wp = wb_ps.tile([P, FT], F32, tag="wout", name="wout")
            nc.tensor.matmul(wp, lhsT=r(woT[:, oc * P:(oc + 1) * P]), rhs=r(onorm),
                             start=True, stop=True)
            y_sb = sm.tile([P, FT], F32, tag="y_sb", name="y_sb", bufs=4)
            nc.vector.tensor_copy(out=y_sb, in_=wp)
            nc.sync.dma_start(out=d["y"][oc * P:(oc + 1) * P, isl], in_=y_sb)


def _build_nc():
    nc = bass.Bass()
    d = {
        "x": nc.dram_tensor("x", [C, NTOK], F32, kind="ExternalInput"),
        "winT": nc.dram_tensor("winT", [C, C], F32, kind="ExternalInput"),
        "b_in": nc.dram_tensor("b_in", [C, 1], F32, kind="ExternalInput"),
        "wqT": nc.dram_tensor("wqT", [C, HDIM], F32, kind="ExternalInput"),
        "bq": nc.dram_tensor("bq", [HDIM, 1], F32, kind="ExternalInput"),
        "wkT": nc.dram_tensor("wkT", [C, HDIM], F32, kind="ExternalInput"),
        "bk": nc.dram_tensor("bk", [HDIM, 1], F32, kind="ExternalInput"),
        "wvT": nc.dram_tensor("wvT", [C, HDIM], F32, kind="ExternalInput"),
        "bv": nc.dram_tensor("bv", [HDIM, 1], F32, kind="ExternalInput"),
        "woT": nc.dram_tensor("woT", [HDIM, C], F32, kind="ExternalInput"),
        "gnw": nc.dram_tensor("gnw", [C, 1], F32, kind="ExternalInput"),
        "gnb": nc.dram_tensor("gnb", [C, 1], F32, kind="ExternalInput"),
        "G": nc.dram_tensor("G", [P, 16], F32, kind="ExternalInput"),
        "GT": nc.dram_tensor("GT", [16, P], F32, kind="ExternalInput"),
        "ident": nc.dram_tensor("ident", [HDIM, HDIM], F32, kind="ExternalInput"),
        "y": nc.dram_tensor("y", [C, NTOK], F32, kind="ExternalOutput"),
    }
    with tile.TileContext(nc) as tc:
        with ExitStack() as ctx:
            _emit(ctx, tc, d)
    _split_matmul_waits(nc)
    return nc


def _split_matmul_waits(nc):
    """Walrus encodes at most ONE hw sync-wait per engine instruction
    (matmul/LDWEIGHTS, tensor_tensor, ...). Move excess waits onto NoOps
    inserted right before the instruction on the same engine, one wait per
    NoOp; the engine executes them in order, preserving semantics."""
    fixed = 0
    for fn in nc.m.functions:
        for blk in fn.blocks:
            insts = blk.instructions
            out = []
            changed = False
            for inst in insts:
                si = inst.sync_info
                if si is not None and si.on_wait and len(si.on_wait) > 1:
                    waits = list(si.on_wait)
                    for w in waits[:-1]:
                        nop = mybir.InstNoOp(
                            name=f"I-waitsplit-{fixed}", ins=[], outs=[])
                        nop.engine = inst.engine
                        nop.sync_info = mybir.SyncInfo(on_wait=[w], on_update=[])
                        out.append(nop)
                        fixed += 1
                    inst.sync_info = mybir.SyncInfo(
                        on_wait=[waits[-1]], on_update=list(si.on_update or []))
                    changed = True
                out.append(inst)
            if changed:
                blk.instructions = out
    return fixed


_CACHE = {}


def _get_nc():
    if "nc" not in _CACHE:
        _CACHE["nc"] = _build_nc()
    return _CACHE["nc"]


def _make_in_maps(x, gn_w, gn_b, w_in, b_in, w_q, b_q, w_k, b_k, w_v, b_v, w_out):
    f32 = lambda a: np.ascontiguousarray(np.asarray(a), dtype=np.float32)
    x = f32(x)
    Gm = np.zeros((P, 16), np.float32)
    Gm[np.arange(P), np.arange(P) // 8] = 1.0
    common = {
        "winT": f32(np.asarray(w_in).T),
        "b_in": f32(b_in).reshape(C, 1),
        "gnw": f32(gn_w).reshape(C, 1),
        "gnb": f32(gn_b).reshape(C, 1),
        "G": Gm,
        "GT": np.ascontiguousarray(Gm.T),
        "ident": np.eye(HDIM, dtype=np.float32),
    }
    in_maps = []
    for core in range(8):
        b, hd = divmod(core, 4)
        sl = slice(hd * HDIM, (hd + 1) * HDIM)
        m = dict(common)
        m["x"] = f32(x[b].reshape(C, NTOK))
        m["wqT"] = f32((np.asarray(w_q)[sl] * 0.125).T)
        m["bq"] = f32(np.asarray(b_q)[sl] * 0.125).reshape(HDIM, 1)
        m["wkT"] = f32(np.asarray(w_k)[sl].T)
        m["bk"] = f32(np.asarray(b_k)[sl]).reshape(HDIM, 1)
        m["wvT"] = f32(np.asarray(w_v)[sl].T)
        m["bv"] = f32(np.asarray(b_v)[sl]).reshape(HDIM, 1)
        m["woT"] = f32(np.asarray(w_out)[:, sl].T)
        in_maps.append(m)
    return in_maps


def kernel(x, gn_w, gn_b, w_in, b_in, w_q, b_q, w_k, b_k, w_v, b_v, w_out, b_out,
           _trace=False):
    nc = _get_nc()
    in_maps = _make_in_maps(x, gn_w, gn_b, w_in, b_in, w_q, b_q, w_k, b_k,
                            w_v, b_v, w_out)
    res = run_bass_kernel_spmd(nc, in_maps, list(range(8)), trace=_trace)
    parts = np.stack([np.asarray(res.results[i]["y"]) for i in range(8)])
    x_np = np.asarray(x, dtype=np.float32)
    out = (parts.reshape(2, 4, C, NTOK).sum(axis=1)
           + np.asarray(b_out, dtype=np.float32).reshape(1, C, 1)
           + x_np.reshape(2, C, NTOK))
    out = out.reshape(x_np.shape).astype(np.float32)
    if _trace:
        return out, res
    return out


# revision 25
# speedup vs baseline: 1.0545x; 1.0013x over previous
"""Trainium2 Bass kernel for a 3D attention block (GroupNorm -> 1x1 conv ->
4-head attention over 4096 tokens -> out-proj -> residual).

Sharding: batch(2) x heads(4) = 8 (b, h) pairs, one per NeuronCore.
Each core computes, for its (b, h):
    hn = GroupNorm(x[b]); h = W_in @ hn + b_in
    q = 0.125*(Wq_h @ h + bq_h); k = Wk_h @ h + bk_h; v = Wv_h @ h + bv_h
    S^T = k^T q (per 128-j chunk);  P = exp(S^T);  out = (P^T-contracted) v
    y_part = Wout[:, h] @ (out / rowsum)
Host sums the 4 per-head partials per batch and adds b_out + x (the unshard
step). All weights are pre-sliced/pre-transposed per core on the host.

Layouts on device (partition dim first):
    x, hn, h  : 2 chunks of (128 ch, 4096 tok)
    q, k, v   : (64 d, 4096 tok)
    vT        : (128 j, 32 chunk, 65) with col 64 = ones (softmax denominator)
    S^T tiles : psum (128 j, 2x512 i) = 2 j-chunks side by side, exp'd by
                one ACT instruction into bf16; QK/PV matmuls in bf16, channel
                matmuls in float32r (both run 1 PE cycle/column vs 4 for fp32)
    out       : psum (65 d', 512 i) accumulated over 32 j-chunks; row 64 is
                the softmax denominator (ones column trick)
"""

import numpy as np
from contextlib import ExitStack

import concourse.bass as bass
import concourse.tile as tile
from concourse import mybir
from concourse.bass_utils import run_bass_kernel_spmd

F32 = mybir.dt.float32
AF = mybir.ActivationFunctionType
OP = mybir.AluOpType

P = 128
C = 256
HDIM = 64
NTOK = 4096
FT = 512               # matmul moving free dim (fp32 psum bank)
NI = NTOK // FT        # 8 i-tiles
NJ = NTOK // P         # 32 j-chunks
PAIR = 2               # j-chunks per score psum tile (2 banks)
NG = NJ // PAIR        # 16 groups per i-tile
EPS = 1e-5


def _emit(ctx: ExitStack, tc: tile.TileContext, d):
    nc = tc.nc
    # fp32 matmuls run at 4 cycles/column; float32r (same bits, different PE
    # datapath) runs at 1 cycle/column when the moving free dim is >= 256.
    r = lambda ap: ap.bitcast(mybir.dt.float32r)

    const = ctx.enter_context(tc.tile_pool(name="const", bufs=1))
    data = ctx.enter_context(tc.tile_pool(name="data", bufs=1))
    sm = ctx.enter_context(tc.tile_pool(name="sm", bufs=2))

    # ---- constant loads -------------------------------------------------
    def cload(tag, shape, src):
        t = const.tile(shape, F32, tag=tag)
        nc.sync.dma_start(out=t, in_=src[:])
        return t

    def wload(tag, shape, src):
        # Matmul weights are staged through a DVE copy: a matmul (LDWEIGHTS)
        # can carry only ONE hw sync-wait, so its operands must not depend on
        # two different engines (DMA + compute). After staging, every matmul
        # weight is DVE-produced.
        stage = cload(tag + "_st", shape, src)
        t = const.tile(shape, F32, tag=tag, name=tag)
        nc.gpsimd.tensor_copy(out=t.bitcast(mybir.dt.float32r), in_=stage)
        return t

    def wload_f32(tag, shape, src):
        stage = cload(tag + "_st", shape, src)
        t = const.tile(shape, F32, tag=tag, name=tag)
        nc.gpsimd.tensor_copy(out=t, in_=stage)
        return t

    x = [data.tile([P, NTOK], F32, tag=f"x{c}", name=f"x{c}") for c in range(2)]
    for c in range(2):
        for w4 in range(4):
            nc.sync.dma_start(out=x[c][:, w4 * 1024:(w4 + 1) * 1024],
                              in_=d["x"][c * P:(c + 1) * P, w4 * 1024:(w4 + 1) * 1024])

    winT = [wload(f"winT{c}", [P, C], d["winT"][c * P:(c + 1) * P, :]) for c in range(2)]
    wqT = [wload(f"wqT{c}", [P, HDIM], d["wqT"][c * P:(c + 1) * P, :]) for c in range(2)]
    wkT = [wload(f"wkT{c}", [P, HDIM], d["wkT"][c * P:(c + 1) * P, :]) for c in range(2)]
    wvT = [wload(f"wvT{c}", [P, HDIM], d["wvT"][c * P:(c + 1) * P, :]) for c in range(2)]
    woT = wload("woT", [HDIM, C], d["woT"])
    b_in = [cload(f"bin{c}", [P, 1], d["b_in"][c * P:(c + 1) * P, :]) for c in range(2)]
    bq = cload("bq", [HDIM, 1], d["bq"])
    bk = cload("bk", [HDIM, 1], d["bk"])
    bv = cload("bv", [HDIM, 1], d["bv"])
    gnw = [cload(f"gnw{c}", [P, 1], d["gnw"][c * P:(c + 1) * P, :]) for c in range(2)]
    gnb = [cload(f"gnb{c}", [P, 1], d["gnb"][c * P:(c + 1) * P, :]) for c in range(2)]
    G = wload_f32("G", [P, 16], d["G"])
    GT = wload_f32("GT", [16, P], d["GT"])
    ident = wload_f32("ident", [HDIM, HDIM], d["ident"])
    eps16 = const.tile([16, 1], F32, tag="eps16", name="eps16")
    nc.vector.memset(eps16, EPS)
    ones64 = const.tile([1, HDIM], F32, tag="ones64", name="ones64")
    nc.vector.memset(ones64, 1.0)
    ones_col = const.tile([P, 1], mybir.dt.bfloat16, tag="ones_col", name="ones_col")
    nc.vector.memset(ones_col, 1.0)


    hn = [data.tile([P, NTOK], F32, tag=f"hn{c}", name=f"hn{c}") for c in range(2)]
    h = [data.tile([P, NTOK], F32, tag=f"h{c}", name=f"h{c}") for c in range(2)]
    q = data.tile([HDIM, NTOK], mybir.dt.bfloat16, tag="q", name="q")
    k = data.tile([HDIM, NTOK], mybir.dt.bfloat16, tag="k", name="k")
    v = data.tile([HDIM, NTOK], F32, tag="v", name="v")
    vT = data.tile([P, NJ, HDIM + 1], mybir.dt.bfloat16, tag="vT", name="vT")

    # ---- prologue psum pools (close before attention) -------------------
    with tc.tile_pool(name="ps_mm", bufs=2, space="PSUM") as ps_mm, \
         tc.tile_pool(name="ps_tr", bufs=2, space="PSUM") as ps_tr, \
         tc.tile_pool(name="ps_st", bufs=4, space="PSUM") as ps_st:

        # ---- GroupNorm ----------------------------------------------------
        for c in range(2):
            stats8 = sm.tile([P, 8, 6], F32, tag="stats8", name="stats8")
            for s in range(8):
                nc.vector.bn_stats(out=stats8[:, s, :], in_=x[c][:, s * FT:(s + 1) * FT])
            mv = sm.tile([P, 2], F32, tag="mv", name="mv")
            nc.vector.bn_aggr(out=mv, in_=stats8)
            # stat2 = [mu_c, E[x^2]_c]
            stat2 = sm.tile([P, 2], F32, tag="stat2", name="stat2")
            nc.vector.tensor_copy(out=stat2[:, 0:1], in_=mv[:, 0:1])
            nc.vector.tensor_mul(out=stat2[:, 1:2], in0=mv[:, 0:1], in1=mv[:, 0:1])
            nc.vector.tensor_add(out=stat2[:, 1:2], in0=stat2[:, 1:2], in1=mv[:, 1:2])
            # group sums (16 groups per chunk)
            ps_g = ps_st.tile([P, 2], F32, tag="st", name="sg")
            nc.tensor.matmul(ps_g[0:16, :], lhsT=G, rhs=stat2, start=True, stop=True)
            sgx = sm.tile([16, 2], F32, tag="sgx", name="sgx")
            nc.vector.tensor_scalar_mul(out=sgx, in0=ps_g[0:16, :], scalar1=0.125)  # /8
            musqg = sm.tile([16, 1], F32, tag="musqg", name="musqg")
            nc.vector.tensor_mul(out=musqg, in0=sgx[:, 0:1], in1=sgx[:, 0:1])
            varg = sm.tile([16, 1], F32, tag="varg", name="varg")
            nc.vector.tensor_tensor(out=varg, in0=sgx[:, 1:2], in1=musqg, op=OP.subtract)
            sd = sm.tile([16, 1], F32, tag="sd", name="sd")
            nc.scalar.activation(out=sd, in_=varg, func=AF.Sqrt, bias=eps16)
            rstd = sm.tile([16, 1], F32, tag="rstd", name="rstd")
            nc.vector.reciprocal(out=rstd, in_=sd)
            gr = sm.tile([16, 2], F32, tag="gr", name="gr")
            nc.vector.tensor_copy(out=gr[:, 0:1], in_=sgx[:, 0:1])
            nc.vector.tensor_copy(out=gr[:, 1:2], in_=rstd)
            ps_ch = ps_st.tile([P, 2], F32, tag="st", name="sch")
            nc.tensor.matmul(ps_ch, lhsT=GT, rhs=gr, start=True, stop=True)
            A = sm.tile([P, 1], F32, tag="A", name="A")
            nc.vector.tensor_mul(out=A, in0=ps_ch[:, 1:2], in1=gnw[c])
            tmp = sm.tile([P, 1], F32, tag="tmp", name="tmp")
            nc.vector.tensor_mul(out=tmp, in0=ps_ch[:, 0:1], in1=A)
            Bv = sm.tile([P, 1], F32, tag="Bv", name="Bv")
            nc.vector.tensor_tensor(out=Bv, in0=gnb[c], in1=tmp, op=OP.subtract)
            for w4 in range(4):
                sl4 = slice(w4 * 1024, (w4 + 1) * 1024)
                nc.vector.tensor_scalar(out=r(hn[c][:, sl4]), in0=x[c][:, sl4],
                                        scalar1=A, scalar2=Bv,
                                        op0=OP.mult, op1=OP.add)

        # ---- h = W_in @ hn + b_in ----------------------------------------
        for oc in range(2):
            for it in range(NI):
                ps = ps_mm.tile([P, FT], F32, tag="mm", name="mm")
                for cc in range(2):
                    nc.tensor.matmul(ps, lhsT=r(winT[cc][:, oc * P:(oc + 1) * P]),
                                     rhs=r(hn[cc][:, it * FT:(it + 1) * FT]),
                                     start=(cc == 0), stop=(cc == 1))
                nc.vector.tensor_scalar_add(out=r(h[oc][:, it * FT:(it + 1) * FT]),
                                            in0=ps, scalar1=b_in[oc])

        # ---- q, k, v -------------------------------------------------------
        for dst, wT, bias in ((q, wqT, bq), (k, wkT, bk), (v, wvT, bv)):
            for it in range(NI):
                ps = ps_mm.tile([P, FT], F32, tag="mm", name="mm")
                for cc in range(2):
                    nc.tensor.matmul(ps[0:HDIM, :], lhsT=r(wT[cc]),
                                     rhs=r(h[cc][:, it * FT:(it + 1) * FT]),
                                     start=(cc == 0), stop=(cc == 1))
                nc.vector.tensor_scalar_add(out=dst[:, it * FT:(it + 1) * FT],
                                            in0=ps[0:HDIM, :], scalar1=bias)

        # ---- vT (with ones column for softmax denominators) ---------------
        nc.vector.tensor_copy(out=vT[:, :, HDIM:HDIM + 1],
                              in_=ones_col.to_broadcast([P, NJ, 1]))
        for jc in range(NJ):
            ps = ps_tr.tile([P, HDIM], F32, tag="tr", name="tr")
            nc.tensor.transpose(out=ps, in_=v[:, jc * P:(jc + 1) * P], identity=ident)
            nc.vector.tensor_copy(out=vT[:, jc, 0:HDIM], in_=ps)

    # ---- attention ------------------------------------------------------
    qk_ps = ctx.enter_context(tc.tile_pool(name="qk_ps", bufs=2, space="PSUM"))
    pv_ps = ctx.enter_context(tc.tile_pool(name="pv_ps", bufs=2, space="PSUM"))
    wb_ps = ctx.enter_context(tc.tile_pool(name="wb_ps", bufs=1, space="PSUM"))
    es_pool = ctx.enter_context(tc.tile_pool(name="es", bufs=4))

    for it in range(NI):
        isl = slice(it * FT, (it + 1) * FT)
        pv = pv_ps.tile([HDIM + 1, FT], F32, tag="pv", name="pv")
        for g in range(NG):
            qk = qk_ps.tile([P, PAIR * FT], F32, tag="qk", name="qk")
            for u in range(PAIR):
                jc = g * PAIR + u
                nc.tensor.matmul(qk[:, u * FT:(u + 1) * FT],
                                 lhsT=k[:, jc * P:(jc + 1) * P], rhs=q[:, isl],
                                 start=True, stop=True)
            es = es_pool.tile([P, PAIR * FT], mybir.dt.bfloat16, tag="es", name="es")
            nc.scalar.activation(out=es, in_=qk, func=AF.Exp)
            for u in range(PAIR):
                jc = g * PAIR + u
                nc.tensor.matmul(pv, lhsT=vT[:, jc, :], rhs=es[:, u * FT:(u + 1) * FT],
                                 start=(jc == 0), stop=(jc == NJ - 1),
                                 skip_group_check=True)
        # normalize: out[d, i] / out[64, i]
        recip = sm.tile([1, FT], F32, tag="recip", name="recip")
        nc.vector.reciprocal(out=recip, in_=pv[HDIM:HDIM + 1, :])
        bc_ps = wb_ps.tile([HDIM, FT], F32, tag="bc", name="bc")
        nc.tensor.matmul(bc_ps, lhsT=ones64, rhs=recip, start=True, stop=True)
        bc = sm.tile([HDIM, FT], F32, tag="bc_sb", name="bc_sb")
        nc.vector.tensor_copy(out=bc, in_=bc_ps)
        onorm = sm.tile([HDIM, FT], F32, tag="onorm", name="onorm")
        nc.vector.tensor_mul(out=r(onorm), in0=pv[0:HDIM, :], in1=bc)
        # y_part = Wout_h @ onorm
        for oc in range(2):
            wp = wb_ps.tile([P, FT], F32, tag="wout", name="wout")
            nc.tensor.matmul(wp, lhsT=r(woT[:, oc * P:(oc + 1) * P]), rhs=r(onorm),
                             start=True, stop=True)
            y_sb = sm.tile([P, FT], F32, tag="y_sb", name="y_sb", bufs=4)
            nc.vector.tensor_copy(out=y_sb, in_=wp)
            nc.sync.dma_start(out=d["y"][oc * P:(oc + 1) * P, isl], in_=y_sb)


def _build_nc():
    nc = bass.Bass()
    d = {
        "x": nc.dram_tensor("x", [C, NTOK], F32, kind="ExternalInput"),
        "winT": nc.dram_tensor("winT", [C, C], F32, kind="ExternalInput"),
        "b_in": nc.dram_tensor("b_in", [C, 1], F32, kind="ExternalInput"),
        "wqT": nc.dram_tensor("wqT", [C, HDIM], F32, kind="ExternalInput"),
        "bq": nc.dram_tensor("bq", [HDIM, 1], F32, kind="ExternalInput"),
        "wkT": nc.dram_tensor("wkT", [C, HDIM], F32, kind="ExternalInput"),
        "bk": nc.dram_tensor("bk", [HDIM, 1], F32, kind="ExternalInput"),
        "wvT": nc.dram_tensor("wvT", [C, HDIM], F32, kind="ExternalInput"),
        "bv": nc.dram_tensor("bv", [HDIM, 1], F32, kind="ExternalInput"),
        "woT": nc.dram_tensor("woT", [HDIM, C], F32, kind="ExternalInput"),
        "gnw": nc.dram_tensor("gnw", [C, 1], F32, kind="ExternalInput"),
        "gnb": nc.dram_tensor("gnb", [C, 1], F32, kind="ExternalInput"),
        "G": nc.dram_tensor("G", [P, 16], F32, kind="ExternalInput"),
        "GT": nc.dram_tensor("GT", [16, P], F32, kind="ExternalInput"),
        "ident": nc.dram_tensor("ident", [HDIM, HDIM], F32, kind="ExternalInput"),
        "y": nc.dram_tensor("y", [C, NTOK], F32, kind="ExternalOutput"),
    }
    with tile.TileContext(nc) as tc:
        with ExitStack() as ctx:
            _emit(ctx, tc, d)
    _split_matmul_waits(nc)
    return nc


def _split_matmul_waits(nc):
    """Walrus encodes at most ONE hw sync-wait per engine instruction
    (matmul/LDWEIGHTS, tensor_tensor, ...). Move excess waits onto NoOps
    inserted right before the instruction on the same engine, one wait per
    NoOp; the engine executes them in order, preserving semantics."""
    fixed = 0
    for fn in nc.m.functions:
        for blk in fn.blocks:
            insts = blk.instructions
            out = []
            changed = False
            for inst in insts:
                si = inst.sync_info
                if si is not None and si.on_wait and len(si.on_wait) > 1:
                    waits = list(si.on_wait)
                    for w in waits[:-1]:
                        nop = mybir.InstNoOp(
                            name=f"I-waitsplit-{fixed}", ins=[], outs=[])
                        nop.engine = inst.engine
                        nop.sync_info = mybir.SyncInfo(on_wait=[w], on_update=[])
                        out.append(nop)
                        fixed += 1
                    inst.sync_info = mybir.SyncInfo(
                        on_wait=[waits[-1]], on_update=list(si.on_update or []))
                    changed = True
                out.append(inst)
            if changed:
                blk.instructions = out
    return fixed


_CACHE = {}


def _get_nc():
    if "nc" not in _CACHE:
        _CACHE["nc"] = _build_nc()
    return _CACHE["nc"]


def _make_in_maps(x, gn_w, gn_b, w_in, b_in, w_q, b_q, w_k, b_k, w_v, b_v, w_out):
    f32 = lambda a: np.ascontiguousarray(np.asarray(a), dtype=np.float32)
    x = f32(x)
    Gm = np.zeros((P, 16), np.float32)
    Gm[np.arange(P), np.arange(P) // 8] = 1.0
    common = {
        "winT": f32(np.asarray(w_in).T),
        "b_in": f32(b_in).reshape(C, 1),
        "gnw": f32(gn_w).reshape(C, 1),
        "gnb": f32(gn_b).reshape(C, 1),
        "G": Gm,
        "GT": np.ascontiguousarray(Gm.T),
        "ident": np.eye(HDIM, dtype=np.float32),
    }
    in_maps = []
    for core in range(8):
        b, hd = divmod(core, 4)
        sl = slice(hd * HDIM, (hd + 1) * HDIM)
        m = dict(common)
        m["x"] = f32(x[b].reshape(C, NTOK))
        m["wqT"] = f32((np.asarray(w_q)[sl] * 0.125).T)
        m["bq"] = f32(np.asarray(b_q)[sl] * 0.125).reshape(HDIM, 1)
        m["wkT"] = f32(np.asarray(w_k)[sl].T)
        m["bk"] = f32(np.asarray(b_k)[sl]).reshape(HDIM, 1)
        m["wvT"] = f32(np.asarray(w_v)[sl].T)
        m["bv"] = f32(np.asarray(b_v)[sl]).reshape(HDIM, 1)
        m["woT"] = f32(np.asarray(w_out)[:, sl].T)
        in_maps.append(m)
    return in_maps


def kernel(x, gn_w, gn_b, w_in, b_in, w_q, b_q, w_k, b_k, w_v, b_v, w_out, b_out,
           _trace=False):
    nc = _get_nc()
    in_maps = _make_in_maps(x, gn_w, gn_b, w_in, b_in, w_q, b_q, w_k, b_k,
                            w_v, b_v, w_out)
    res = run_bass_kernel_spmd(nc, in_maps, list(range(8)), trace=_trace)
    parts = np.stack([np.asarray(res.results[i]["y"]) for i in range(8)])
    x_np = np.asarray(x, dtype=np.float32)
    out = (parts.reshape(2, 4, C, NTOK).sum(axis=1)
           + np.asarray(b_out, dtype=np.float32).reshape(1, C, 1)
           + x_np.reshape(2, C, NTOK))
    out = out.reshape(x_np.shape).astype(np.float32)
    if _trace:
        return out, res
    return out


# revision 26
# speedup vs baseline: 1.0651x; 1.0101x over previous
"""Trainium2 Bass kernel for a 3D attention block (GroupNorm -> 1x1 conv ->
4-head attention over 4096 tokens -> out-proj -> residual).

Sharding: batch(2) x heads(4) = 8 (b, h) pairs, one per NeuronCore.
Each core computes, for its (b, h):
    hn = GroupNorm(x[b]); h = W_in @ hn + b_in
    q = 0.125*(Wq_h @ h + bq_h); k = Wk_h @ h + bk_h; v = Wv_h @ h + bv_h
    S^T = k^T q (per 128-j chunk);  P = exp(S^T);  out = (P^T-contracted) v
    y_part = Wout[:, h] @ (out / rowsum)
Host sums the 4 per-head partials per batch and adds b_out + x (the unshard
step). All weights are pre-sliced/pre-transposed per core on the host.

Layouts on device (partition dim first):
    x, hn, h  : 2 chunks of (128 ch, 4096 tok)
    q, k, v   : (64 d, 4096 tok)
    vT        : (128 j, 32 chunk, 65) with col 64 = ones (softmax denominator)
    S^T tiles : psum (128 j, 2x512 i) = 2 j-chunks side by side, exp'd by
                one ACT instruction into bf16; QK/PV matmuls in bf16, channel
                matmuls in float32r (both run 1 PE cycle/column vs 4 for fp32)
    out       : psum (65 d', 512 i) accumulated over 32 j-chunks; row 64 is
                the softmax denominator (ones column trick)
"""

import numpy as np
from contextlib import ExitStack

import concourse.bass as bass
import concourse.tile as tile
from concourse import mybir
from concourse.bass_utils import run_bass_kernel_spmd

F32 = mybir.dt.float32
AF = mybir.ActivationFunctionType
OP = mybir.AluOpType

P = 128
C = 256
HDIM = 64
NTOK = 4096
FT = 512               # matmul moving free dim (fp32 psum bank)
NI = NTOK // FT        # 8 i-tiles
NJ = NTOK // P         # 32 j-chunks
PAIR = 2               # j-chunks per score psum tile (2 banks)
NG = NJ // PAIR        # 16 groups per i-tile
EPS = 1e-5


def _emit(ctx: ExitStack, tc: tile.TileContext, d):
    nc = tc.nc
    # fp32 matmuls run at 4 cycles/column; float32r (same bits, different PE
    # datapath) runs at 1 cycle/column when the moving free dim is >= 256.
    r = lambda ap: ap.bitcast(mybir.dt.float32r)

    const = ctx.enter_context(tc.tile_pool(name="const", bufs=1))
    data = ctx.enter_context(tc.tile_pool(name="data", bufs=1))
    sm = ctx.enter_context(tc.tile_pool(name="sm", bufs=3))

    # ---- constant loads -------------------------------------------------
    def cload(tag, shape, src):
        t = const.tile(shape, F32, tag=tag)
        nc.sync.dma_start(out=t, in_=src[:])
        return t

    def wload(tag, shape, src):
        # Matmul weights are staged through a DVE copy: a matmul (LDWEIGHTS)
        # can carry only ONE hw sync-wait, so its operands must not depend on
        # two different engines (DMA + compute). After staging, every matmul
        # weight is DVE-produced.
        stage = cload(tag + "_st", shape, src)
        t = const.tile(shape, F32, tag=tag, name=tag)
        nc.gpsimd.tensor_copy(out=t.bitcast(mybir.dt.float32r), in_=stage)
        return t

    def wload_f32(tag, shape, src):
        stage = cload(tag + "_st", shape, src)
        t = const.tile(shape, F32, tag=tag, name=tag)
        nc.gpsimd.tensor_copy(out=t, in_=stage)
        return t

    x = [data.tile([P, NTOK], F32, tag=f"x{c}", name=f"x{c}") for c in range(2)]
    for c in range(2):
        for w4 in range(4):
            nc.sync.dma_start(out=x[c][:, w4 * 1024:(w4 + 1) * 1024],
                              in_=d["x"][c * P:(c + 1) * P, w4 * 1024:(w4 + 1) * 1024])

    winT = [wload(f"winT{c}", [P, C], d["winT"][c * P:(c + 1) * P, :]) for c in range(2)]
    wqT = [wload(f"wqT{c}", [P, HDIM], d["wqT"][c * P:(c + 1) * P, :]) for c in range(2)]
    wkT = [wload(f"wkT{c}", [P, HDIM], d["wkT"][c * P:(c + 1) * P, :]) for c in range(2)]
    wvT = [wload(f"wvT{c}", [P, HDIM], d["wvT"][c * P:(c + 1) * P, :]) for c in range(2)]
    woT = wload("woT", [HDIM, C], d["woT"])
    b_in = [cload(f"bin{c}", [P, 1], d["b_in"][c * P:(c + 1) * P, :]) for c in range(2)]
    bq = cload("bq", [HDIM, 1], d["bq"])
    bk = cload("bk", [HDIM, 1], d["bk"])
    bv = cload("bv", [HDIM, 1], d["bv"])
    gnw = [cload(f"gnw{c}", [P, 1], d["gnw"][c * P:(c + 1) * P, :]) for c in range(2)]
    gnb = [cload(f"gnb{c}", [P, 1], d["gnb"][c * P:(c + 1) * P, :]) for c in range(2)]
    G = wload_f32("G", [P, 16], d["G"])
    GT = wload_f32("GT", [16, P], d["GT"])
    ident = wload_f32("ident", [HDIM, HDIM], d["ident"])
    eps16 = const.tile([16, 1], F32, tag="eps16", name="eps16")
    nc.vector.memset(eps16, EPS)
    ones64 = const.tile([1, HDIM], F32, tag="ones64", name="ones64")
    nc.vector.memset(ones64, 1.0)
    ones_col = const.tile([P, 1], mybir.dt.bfloat16, tag="ones_col", name="ones_col")
    nc.vector.memset(ones_col, 1.0)


    hn = [data.tile([P, NTOK], F32, tag=f"hn{c}", name=f"hn{c}") for c in range(2)]
    h = [data.tile([P, NTOK], F32, tag=f"h{c}", name=f"h{c}") for c in range(2)]
    q = data.tile([HDIM, NTOK], mybir.dt.bfloat16, tag="q", name="q")
    k = data.tile([HDIM, NTOK], mybir.dt.bfloat16, tag="k", name="k")
    v = data.tile([HDIM, NTOK], F32, tag="v", name="v")
    vT = data.tile([P, NJ, HDIM + 1], mybir.dt.bfloat16, tag="vT", name="vT")

    # ---- prologue psum pools (close before attention) -------------------
    with tc.tile_pool(name="ps_mm", bufs=2, space="PSUM") as ps_mm, \
         tc.tile_pool(name="ps_tr", bufs=2, space="PSUM") as ps_tr, \
         tc.tile_pool(name="ps_st", bufs=4, space="PSUM") as ps_st:

        # ---- GroupNorm ----------------------------------------------------
        for c in range(2):
            stats8 = sm.tile([P, 8, 6], F32, tag="stats8", name="stats8")
            for s in range(8):
                nc.vector.bn_stats(out=stats8[:, s, :], in_=x[c][:, s * FT:(s + 1) * FT])
            mv = sm.tile([P, 2], F32, tag="mv", name="mv")
            nc.vector.bn_aggr(out=mv, in_=stats8)
            # stat2 = [mu_c, E[x^2]_c]
            stat2 = sm.tile([P, 2], F32, tag="stat2", name="stat2")
            nc.vector.tensor_copy(out=stat2[:, 0:1], in_=mv[:, 0:1])
            nc.vector.tensor_mul(out=stat2[:, 1:2], in0=mv[:, 0:1], in1=mv[:, 0:1])
            nc.vector.tensor_add(out=stat2[:, 1:2], in0=stat2[:, 1:2], in1=mv[:, 1:2])
            # group sums (16 groups per chunk)
            ps_g = ps_st.tile([P, 2], F32, tag="st", name="sg")
            nc.tensor.matmul(ps_g[0:16, :], lhsT=G, rhs=stat2, start=True, stop=True)
            sgx = sm.tile([16, 2], F32, tag="sgx", name="sgx")
            nc.vector.tensor_scalar_mul(out=sgx, in0=ps_g[0:16, :], scalar1=0.125)  # /8
            musqg = sm.tile([16, 1], F32, tag="musqg", name="musqg")
            nc.vector.tensor_mul(out=musqg, in0=sgx[:, 0:1], in1=sgx[:, 0:1])
            varg = sm.tile([16, 1], F32, tag="varg", name="varg")
            nc.vector.tensor_tensor(out=varg, in0=sgx[:, 1:2], in1=musqg, op=OP.subtract)
            sd = sm.tile([16, 1], F32, tag="sd", name="sd")
            nc.scalar.activation(out=sd, in_=varg, func=AF.Sqrt, bias=eps16)
            rstd = sm.tile([16, 1], F32, tag="rstd", name="rstd")
            nc.vector.reciprocal(out=rstd, in_=sd)
            gr = sm.tile([16, 2], F32, tag="gr", name="gr")
            nc.vector.tensor_copy(out=gr[:, 0:1], in_=sgx[:, 0:1])
            nc.vector.tensor_copy(out=gr[:, 1:2], in_=rstd)
            ps_ch = ps_st.tile([P, 2], F32, tag="st", name="sch")
            nc.tensor.matmul(ps_ch, lhsT=GT, rhs=gr, start=True, stop=True)
            A = sm.tile([P, 1], F32, tag="A", name="A")
            nc.vector.tensor_mul(out=A, in0=ps_ch[:, 1:2], in1=gnw[c])
            tmp = sm.tile([P, 1], F32, tag="tmp", name="tmp")
            nc.vector.tensor_mul(out=tmp, in0=ps_ch[:, 0:1], in1=A)
            Bv = sm.tile([P, 1], F32, tag="Bv", name="Bv")
            nc.vector.tensor_tensor(out=Bv, in0=gnb[c], in1=tmp, op=OP.subtract)
            for w4 in range(4):
                sl4 = slice(w4 * 1024, (w4 + 1) * 1024)
                nc.vector.tensor_scalar(out=r(hn[c][:, sl4]), in0=x[c][:, sl4],
                                        scalar1=A, scalar2=Bv,
                                        op0=OP.mult, op1=OP.add)

        # ---- h = W_in @ hn + b_in ----------------------------------------
        for oc in range(2):
            for it in range(NI):
                ps = ps_mm.tile([P, FT], F32, tag="mm", name="mm")
                for cc in range(2):
                    nc.tensor.matmul(ps, lhsT=r(winT[cc][:, oc * P:(oc + 1) * P]),
                                     rhs=r(hn[cc][:, it * FT:(it + 1) * FT]),
                                     start=(cc == 0), stop=(cc == 1))
                nc.vector.tensor_scalar_add(out=r(h[oc][:, it * FT:(it + 1) * FT]),
                                            in0=ps, scalar1=b_in[oc])

        # ---- q, k, v -------------------------------------------------------
        for dst, wT, bias in ((q, wqT, bq), (k, wkT, bk), (v, wvT, bv)):
            for it in range(NI):
                ps = ps_mm.tile([P, FT], F32, tag="mm", name="mm")
                for cc in range(2):
                    nc.tensor.matmul(ps[0:HDIM, :], lhsT=r(wT[cc]),
                                     rhs=r(h[cc][:, it * FT:(it + 1) * FT]),
                                     start=(cc == 0), stop=(cc == 1))
                nc.scalar.add(out=dst[:, it * FT:(it + 1) * FT],
                              in_=ps[0:HDIM, :], add=bias)

        # ---- vT (with ones column for softmax denominators) ---------------
        nc.vector.tensor_copy(out=vT[:, :, HDIM:HDIM + 1],
                              in_=ones_col.to_broadcast([P, NJ, 1]))
        for jc in range(NJ):
            ps = ps_tr.tile([P, HDIM], F32, tag="tr", name="tr")
            nc.tensor.transpose(out=ps, in_=v[:, jc * P:(jc + 1) * P], identity=ident)
            nc.vector.tensor_copy(out=vT[:, jc, 0:HDIM], in_=ps)

    # ---- attention ------------------------------------------------------
    qk_ps = ctx.enter_context(tc.tile_pool(name="qk_ps", bufs=2, space="PSUM"))
    pv_ps = ctx.enter_context(tc.tile_pool(name="pv_ps", bufs=2, space="PSUM"))
    wb_ps = ctx.enter_context(tc.tile_pool(name="wb_ps", bufs=1, space="PSUM"))
    es_pool = ctx.enter_context(tc.tile_pool(name="es", bufs=6))

    for it in range(NI):
        isl = slice(it * FT, (it + 1) * FT)
        pv = pv_ps.tile([HDIM + 1, FT], F32, tag="pv", name="pv")
        for g in range(NG):
            qk = qk_ps.tile([P, PAIR * FT], F32, tag="qk", name="qk")
            for u in range(PAIR):
                jc = g * PAIR + u
                nc.tensor.matmul(qk[:, u * FT:(u + 1) * FT],
                                 lhsT=k[:, jc * P:(jc + 1) * P], rhs=q[:, isl],
                                 start=True, stop=True)
            es = es_pool.tile([P, PAIR * FT], mybir.dt.bfloat16, tag="es", name="es")
            nc.scalar.activation(out=es, in_=qk, func=AF.Exp)
            for u in range(PAIR):
                jc = g * PAIR + u
                nc.tensor.matmul(pv, lhsT=vT[:, jc, :], rhs=es[:, u * FT:(u + 1) * FT],
                                 start=(jc == 0), stop=(jc == NJ - 1),
                                 skip_group_check=True)
        # normalize: out[d, i] / out[64, i]
        recip = sm.tile([1, FT], F32, tag="recip", name="recip")
        nc.vector.reciprocal(out=recip, in_=pv[HDIM:HDIM + 1, :])
        bc_ps = wb_ps.tile([HDIM, FT], F32, tag="bc", name="bc")
        nc.tensor.matmul(bc_ps, lhsT=ones64, rhs=recip, start=True, stop=True)
        bc = sm.tile([HDIM, FT], F32, tag="bc_sb", name="bc_sb")
        nc.vector.tensor_copy(out=bc, in_=bc_ps)
        onorm = sm.tile([HDIM, FT], F32, tag="onorm", name="onorm")
        nc.vector.tensor_mul(out=r(onorm), in0=pv[0:HDIM, :], in1=bc)
        # y_part = Wout_h @ onorm
        for oc in range(2):
            wp = wb_ps.tile([P, FT], F32, tag="wout", name="wout")
            nc.tensor.matmul(wp, lhsT=r(woT[:, oc * P:(oc + 1) * P]), rhs=r(onorm),
                             start=True, stop=True)
            y_sb = sm.tile([P, FT], F32, tag="y_sb", name="y_sb", bufs=4)
            nc.vector.tensor_copy(out=y_sb, in_=wp)
            nc.sync.dma_start(out=d["y"][oc * P:(oc + 1) * P, isl], in_=y_sb)


def _build_nc():
    nc = bass.Bass()
    d = {
        "x": nc.dram_tensor("x", [C, NTOK], F32, kind="ExternalInput"),
        "winT": nc.dram_tensor("winT", [C, C], F32, kind="ExternalInput"),
        "b_in": nc.dram_tensor("b_in", [C, 1], F32, kind="ExternalInput"),
        "wqT": nc.dram_tensor("wqT", [C, HDIM], F32, kind="ExternalInput"),
        "bq": nc.dram_tensor("bq", [HDIM, 1], F32, kind="ExternalInput"),
        "wkT": nc.dram_tensor("wkT", [C, HDIM], F32, kind="ExternalInput"),
        "bk": nc.dram_tensor("bk", [HDIM, 1], F32, kind="ExternalInput"),
        "wvT": nc.dram_tensor("wvT", [C, HDIM], F32, kind="ExternalInput"),
        "bv": nc.dram_tensor("bv", [HDIM, 1], F32, kind="ExternalInput"),
        "woT": nc.dram_tensor("woT", [HDIM, C], F32, kind="ExternalInput"),
        "gnw": nc.dram_tensor("gnw", [C, 1], F32, kind="ExternalInput"),
        "gnb": nc.dram_tensor("gnb", [C, 1], F32, kind="ExternalInput"),
        "G": nc.dram_tensor("G", [P, 16], F32, kind="ExternalInput"),
        "GT": nc.dram_tensor("GT", [16, P], F32, kind="ExternalInput"),
        "ident": nc.dram_tensor("ident", [HDIM, HDIM], F32, kind="ExternalInput"),
        "y": nc.dram_tensor("y", [C, NTOK], F32, kind="ExternalOutput"),
    }
    with tile.TileContext(nc) as tc:
        with ExitStack() as ctx:
            _emit(ctx, tc, d)
    _split_matmul_waits(nc)
    return nc


def _split_matmul_waits(nc):
    """Walrus encodes at most ONE hw sync-wait per engine instruction
    (matmul/LDWEIGHTS, tensor_tensor, ...). Move excess waits onto NoOps
    inserted right before the instruction on the same engine, one wait per
    NoOp; the engine executes them in order, preserving semantics."""
    fixed = 0
    for fn in nc.m.functions:
        for blk in fn.blocks:
            insts = blk.instructions
            out = []
            changed = False
            for inst in insts:
                si = inst.sync_info
                if si is not None and si.on_wait and len(si.on_wait) > 1:
                    waits = list(si.on_wait)
                    for w in waits[:-1]:
                        nop = mybir.InstNoOp(
                            name=f"I-waitsplit-{fixed}", ins=[], outs=[])
                        nop.engine = inst.engine
                        nop.sync_info = mybir.SyncInfo(on_wait=[w], on_update=[])
                        out.append(nop)
                        fixed += 1
                    inst.sync_info = mybir.SyncInfo(
                        on_wait=[waits[-1]], on_update=list(si.on_update or []))
                    changed = True
                out.append(inst)
            if changed:
                blk.instructions = out
    return fixed


_CACHE = {}


def _get_nc():
    if "nc" not in _CACHE:
        _CACHE["nc"] = _build_nc()
    return _CACHE["nc"]


def _make_in_maps(x, gn_w, gn_b, w_in, b_in, w_q, b_q, w_k, b_k, w_v, b_v, w_out):
    f32 = lambda a: np.ascontiguousarray(np.asarray(a), dtype=np.float32)
    x = f32(x)
    Gm = np.zeros((P, 16), np.float32)
    Gm[np.arange(P), np.arange(P) // 8] = 1.0
    common = {
        "winT": f32(np.asarray(w_in).T),
        "b_in": f32(b_in).reshape(C, 1),
        "gnw": f32(gn_w).reshape(C, 1),
        "gnb": f32(gn_b).reshape(C, 1),
        "G": Gm,
        "GT": np.ascontiguousarray(Gm.T),
        "ident": np.eye(HDIM, dtype=np.float32),
    }
    in_maps = []
    for core in range(8):
        b, hd = divmod(core, 4)
        sl = slice(hd * HDIM, (hd + 1) * HDIM)
        m = dict(common)
        m["x"] = f32(x[b].reshape(C, NTOK))
        m["wqT"] = f32((np.asarray(w_q)[sl] * 0.125).T)
        m["bq"] = f32(np.asarray(b_q)[sl] * 0.125).reshape(HDIM, 1)
        m["wkT"] = f32(np.asarray(w_k)[sl].T)
        m["bk"] = f32(np.asarray(b_k)[sl]).reshape(HDIM, 1)
        m["wvT"] = f32(np.asarray(w_v)[sl].T)
        m["bv"] = f32(np.asarray(b_v)[sl]).reshape(HDIM, 1)
        m["woT"] = f32(np.asarray(w_out)[:, sl].T)
        in_maps.append(m)
    return in_maps


def kernel(x, gn_w, gn_b, w_in, b_in, w_q, b_q, w_k, b_k, w_v, b_v, w_out, b_out,
           _trace=False):
    nc = _get_nc()
    in_maps = _make_in_maps(x, gn_w, gn_b, w_in, b_in, w_q, b_q, w_k, b_k,
                            w_v, b_v, w_out)
    res = run_bass_kernel_spmd(nc, in_maps, list(range(8)), trace=_trace)
    parts = np.stack([np.asarray(res.results[i]["y"]) for i in range(8)])
    x_np = np.asarray(x, dtype=np.float32)
    out = (parts.reshape(2, 4, C, NTOK).sum(axis=1)
           + np.asarray(b_out, dtype=np.float32).reshape(1, C, 1)
           + x_np.reshape(2, C, NTOK))
    out = out.reshape(x_np.shape).astype(np.float32)
    if _trace:
        return out, res
    return out
